# revision 1
# baseline (speedup 1.0000x reference)
"""BertAttention (QKV proj + MHA + out-proj + residual + LayerNorm) on 8
Trainium2 NeuronCores.

Sharding: tensor-parallel over heads. Core c owns heads {2c, 2c+1} (a
128-wide slice of the hidden dim): it computes Q/K/V projections for its
slice over the full batch*seq, runs attention for its 8 (batch, head)
pairs entirely out of SBUF, then an AllToAll re-shards the attention
context from head-split to sequence-split so each core runs the output
projection + residual + LayerNorm on its 1024-row shard of the flattened
(B*S) dimension. Host concatenates the 8 row-shards.

Matmuls run in bf16 (fp32 PSUM accumulate); softmax and LayerNorm
arithmetic stay fp32. The attention-path error this introduces is
suppressed ~100x in the final output by the fp32 residual.

softmax: scores are built transposed (scoresT[k, q] per head) so the
probs@V contraction needs no transpose; the row-sum comes from an extra
all-ones column appended to V; the attention mask enters as
exp(mask[b,k]) folded into V's rows and the ones column (exp(s+m) =
exp(s)*exp(m), and mask only depends on the key position).
"""

import os
import sys
import contextlib
import ctypes
import types

import numpy as np
import ml_dtypes

N_CORES = 8
B, S, H = 4, 2048, 1024
NH, DH = 16, 64
R = B * S            # 8192 flattened rows
RS = R // N_CORES    # 1024 rows per core (output shard)
HB = H // N_CORES    # 128 head-dim columns per core (2 heads)
SBW = 512            # seq-block width for projections
NSB = R // SBW       # 16 seq blocks
NHC = H // 128       # 8 contraction chunks of 128
NKB = S // 128       # 16 key blocks per batch
LN_EPS = 1e-12

last_exec_time_ns = None

# ---------------------------------------------------------------------------
# NTFF profile hook shim (axon images without antenv.axon_hooks).
# Only needed when tracing; harmless otherwise.
_SO_PATH = "/opt/axon/libaxon_pjrt.so"


def _install_ntff_shim():
    try:
        from antenv import axon_hooks  # noqa: F401
        return
    except ImportError:
        pass
    hook = None
    try:
        lib = ctypes.CDLL(_SO_PATH)
        if hasattr(lib, "axon_start_nrt_profile"):
            lib.axon_start_nrt_profile.argtypes = [
                ctypes.POINTER(ctypes.c_int64), ctypes.c_size_t]
            lib.axon_start_nrt_profile.restype = ctypes.c_int64
            lib.axon_stop_nrt_profile.argtypes = [ctypes.c_char_p]
            lib.axon_stop_nrt_profile.restype = ctypes.c_int64

            @contextlib.contextmanager
            def _hook(output_dir, device_ids):
                import jax
                jax.devices()
                if device_ids:
                    ids = (ctypes.c_int64 * len(device_ids))(*device_ids)
                    rc = lib.axon_start_nrt_profile(ids, len(device_ids))
                else:
                    rc = lib.axon_start_nrt_profile(None, 0)
                if rc != 0:
                    raise RuntimeError(f"axon_start_nrt_profile rc={rc}")
                try:
                    yield
                finally:
                    n = lib.axon_stop_nrt_profile(str(output_dir).encode())
                    print(f"profile: {n} ntff file(s) in {output_dir}",
                          file=sys.stderr)

            hook = _hook
    except OSError:
        pass
    mod = types.ModuleType("antenv.axon_hooks")
    mod._hook = hook
    mod.get_axon_ntff_profile_hook = lambda: mod._hook
    mod.set_axon_ntff_profile_hook = lambda h: setattr(mod, "_hook", h)
    sys.modules["antenv.axon_hooks"] = mod
    try:
        import antenv
        antenv.axon_hooks = mod
    except ImportError:
        pass


# ---------------------------------------------------------------------------

def _build(dbg=False):
    from concourse import bacc, tile
    import concourse.mybir as mybir

    f32 = mybir.dt.float32
    bf16 = mybir.dt.bfloat16
    AF = mybir.ActivationFunctionType
    ALU = mybir.AluOpType

    nc = bacc.Bacc("TRN2", target_bir_lowering=False, debug=False,
                   num_devices=N_CORES)

    # ---- DRAM I/O ----
    hT_d = nc.dram_tensor("hT", [H, R], bf16, kind="ExternalInput")
    wqT_d = nc.dram_tensor("wqT", [H, HB], bf16, kind="ExternalInput")
    wkT_d = nc.dram_tensor("wkT", [H, HB], bf16, kind="ExternalInput")
    wvT_d = nc.dram_tensor("wvT", [H, HB], bf16, kind="ExternalInput")
    woT_d = nc.dram_tensor("woT", [H, H], bf16, kind="ExternalInput")
    bq_d = nc.dram_tensor("bq", [HB], f32, kind="ExternalInput")
    bk_d = nc.dram_tensor("bk", [HB], f32, kind="ExternalInput")
    bv_d = nc.dram_tensor("bv", [HB], f32, kind="ExternalInput")
    bo_d = nc.dram_tensor("bo", [H], f32, kind="ExternalInput")
    gamma_d = nc.dram_tensor("gamma", [H], f32, kind="ExternalInput")
    beta_d = nc.dram_tensor("beta", [H], f32, kind="ExternalInput")
    maskT_d = nc.dram_tensor("maskT", [S, B], f32, kind="ExternalInput")
    hres_d = nc.dram_tensor("hres", [RS, H], f32, kind="ExternalInput")
    out_d = nc.dram_tensor("out", [RS, H], f32, kind="ExternalOutput")
    if dbg:
        dbg_qt = nc.dram_tensor("dbg_qt", [128, NSB, SBW], f32,
                                kind="ExternalOutput")
        dbg_kt = nc.dram_tensor("dbg_kt", [128, NSB, SBW], f32,
                                kind="ExternalOutput")
        dbg_va = nc.dram_tensor("dbg_va", [2, 128, NKB, DH + 1], f32,
                                kind="ExternalOutput")
        dbg_ctxT = nc.dram_tensor("dbg_ctxT", [128, NSB, SBW], f32,
                                  kind="ExternalOutput")
        dbg_ctxF = nc.dram_tensor("dbg_ctxF", [128, NHC, RS], f32,
                                  kind="ExternalOutput")

    with tile.TileContext(nc) as tc:
        with (
            tc.tile_pool(name="const", bufs=1) as cpool,
            tc.tile_pool(name="psA", bufs=2, space="PSUM") as psA,
            tc.tile_pool(name="psB", bufs=2, space="PSUM") as psB,
            tc.tile_pool(name="psC", bufs=2, space="PSUM") as psC,
            tc.tile_pool(name="dram", bufs=1, space="DRAM") as dpool,
        ):
            # ================= setup =================
            wq_sb = cpool.tile([128, NHC, HB], bf16, tag="wq")
            wk_sb = cpool.tile([128, NHC, HB], bf16, tag="wk")
            wv_sb = cpool.tile([128, NHC, HB], bf16, tag="wv")
            for c in range(NHC):
                nc.sync.dma_start(wq_sb[:, c, :], wqT_d[128 * c:128 * (c + 1), :])
                nc.sync.dma_start(wk_sb[:, c, :], wkT_d[128 * c:128 * (c + 1), :])
                nc.sync.dma_start(wv_sb[:, c, :], wvT_d[128 * c:128 * (c + 1), :])

            bq_sb = cpool.tile([128, 1], f32, tag="bq")
            bk_sb = cpool.tile([128, 1], f32, tag="bk")
            nc.sync.dma_start(bq_sb[:, :], bq_d[:].unsqueeze(1))
            nc.sync.dma_start(bk_sb[:, :], bk_d[:].unsqueeze(1))

            # bv broadcast along partitions (V is in [seq, d] layout)
            bv_b = cpool.tile([128, HB], f32, tag="bv_b")
            nc.sync.dma_start(bv_b[:, :],
                              bv_d[:].unsqueeze(0).partition_broadcast(128))

            # exp(mask), laid out [k-in-block, kblock, batch]
            em_sb = cpool.tile([128, NKB, B], f32, tag="em")
            for kb in range(NKB):
                nc.sync.dma_start(em_sb[:, kb, :],
                                  maskT_d[128 * kb:128 * (kb + 1), :])
            nc.scalar.activation(em_sb[:, :, :], em_sb[:, :, :], AF.Exp)

            # attention context, persistent until the AllToAll
            ctxT_sb = cpool.tile([128, NSB, SBW], bf16, tag="ctxT")

            # out-proj weights / residual / LN params: loaded early so the
            # DMAs and the bo-fold overlap the attention phase.
            oload = tc.alloc_tile_pool(name="oload", bufs=1)
            wo_sb = oload.tile([128, NHC, H], bf16, tag="wo")
            for c in range(NHC):
                nc.sync.dma_start(wo_sb[:, c, :],
                                  woT_d[128 * c:128 * (c + 1), :])
            hres_sb = oload.tile([128, RS // 128, H], f32, tag="hres")
            for t in range(RS // 128):
                nc.sync.dma_start(hres_sb[:, t, :],
                                  hres_d[128 * t:128 * (t + 1), :])
            bo_b = oload.tile([128, H], f32, tag="bo_b")
            gamma_b = oload.tile([128, H], f32, tag="gamma_b")
            beta_b = oload.tile([128, H], f32, tag="beta_b")
            nc.sync.dma_start(
                bo_b[:, :], bo_d[:].unsqueeze(0).partition_broadcast(128))
            nc.sync.dma_start(
                gamma_b[:, :],
                gamma_d[:].unsqueeze(0).partition_broadcast(128))
            nc.sync.dma_start(
                beta_b[:, :],
                beta_d[:].unsqueeze(0).partition_broadcast(128))
            for t in range(RS // 128):
                nc.vector.tensor_add(hres_sb[:, t, :], hres_sb[:, t, :],
                                     bo_b[:, :])

            # A2A staging buffers (DRAM)
            a2a_in = dpool.tile([N_CORES, 128, RS], bf16, tag="a2a_in")
            a2a_out = dpool.tile([N_CORES, 128, RS], bf16, tag="a2a_out")

            # ============ phases 1+2: projections + attention =============
            with (
                tc.tile_pool(name="attn", bufs=2) as apool,
                tc.tile_pool(name="ptp", bufs=3) as ptpool,
            ):
                qt_sb = apool.tile([128, NSB, SBW], bf16, tag="qt", bufs=1)
                kt_sb = apool.tile([128, NSB, SBW], bf16, tag="kt", bufs=1)
                for b in range(B):
                    va = [apool.tile([128, NKB, DH + 1], bf16, tag=f"va{h}",
                                     name=f"va{h}")
                          for h in range(2)]
                    for i in range(4 * b, 4 * b + 4):
                        # hidden^T block [H, SBW] -> [128, NHC, SBW]
                        hsb = apool.tile([128, NHC, SBW], bf16, tag="hsb",
                                         bufs=3)
                        for c in range(NHC):
                            nc.sync.dma_start(
                                hsb[:, c, :],
                                hT_d[128 * c:128 * (c + 1),
                                     SBW * i:SBW * (i + 1)])
                        # Q^T block
                        pq = psA.tile([128, SBW], f32, tag="proj")
                        for c in range(NHC):
                            nc.tensor.matmul(pq[:, :], wq_sb[:, c, :],
                                             hsb[:, c, :],
                                             start=(c == 0),
                                             stop=(c == NHC - 1))
                        nc.vector.tensor_scalar_add(qt_sb[:, i, :], pq[:, :],
                                                    bq_sb[:, :])
                        # K^T block
                        pk = psA.tile([128, SBW], f32, tag="proj")
                        for c in range(NHC):
                            nc.tensor.matmul(pk[:, :], wk_sb[:, c, :],
                                             hsb[:, c, :],
                                             start=(c == 0),
                                             stop=(c == NHC - 1))
                        nc.vector.tensor_scalar_add(kt_sb[:, i, :], pk[:, :],
                                                    bk_sb[:, :])
                        # V in natural [seq, d] layout, 4 sub-blocks of 128
                        for sub in range(4):
                            kb = 4 * (i - 4 * b) + sub  # key block in batch
                            pv = psA.tile([128, SBW], f32, tag="proj")
                            for c in range(NHC):
                                nc.tensor.matmul(
                                    pv[:, 0:HB],
                                    hsb[:, c, 128 * sub:128 * (sub + 1)],
                                    wv_sb[:, c, :],
                                    start=(c == 0), stop=(c == NHC - 1))
                            emcol = em_sb[:, kb, b].unsqueeze(1)
                            t1 = apool.tile([128, HB], f32, tag="t1")
                            nc.vector.tensor_add(t1[:, :], pv[:, 0:HB],
                                                 bv_b[:, :])
                            for h in range(2):
                                nc.vector.tensor_scalar_mul(
                                    va[h][:, kb, 0:DH],
                                    t1[:, DH * h:DH * (h + 1)], emcol)
                                nc.vector.tensor_copy(va[h][:, kb, DH:DH + 1],
                                                      emcol)

                    # ---- attention for batch b ----
                    for qg in range(4):
                        blk = 4 * b + qg
                        pc0 = psC.tile([DH + 1, SBW], f32, tag="ctx")
                        pc1 = psC.tile([DH + 1, SBW], f32, tag="ctx")
                        for kb in range(NKB):
                            sblk = 4 * b + kb // 4
                            kcol = 128 * (kb % 4)
                            sc = psB.tile([128, 2 * SBW], f32, tag="sc")
                            nc.tensor.matmul(
                                sc[:, 0:SBW],
                                kt_sb[0:DH, sblk, kcol:kcol + 128],
                                qt_sb[0:DH, blk, :],
                                start=True, stop=True)
                            nc.tensor.matmul(
                                sc[:, SBW:2 * SBW],
                                kt_sb[DH:2 * DH, sblk, kcol:kcol + 128],
                                qt_sb[DH:2 * DH, blk, :],
                                start=True, stop=True)
                            pt = ptpool.tile([128, 2 * SBW], bf16, tag="pt")
                            nc.scalar.activation(pt[:, :], sc[:, :], AF.Exp,
                                                 scale=0.125)
                            nc.tensor.matmul(pc0[:, :], va[0][:, kb, :],
                                             pt[:, 0:SBW],
                                             start=(kb == 0),
                                             stop=(kb == NKB - 1))
                            nc.tensor.matmul(pc1[:, :], va[1][:, kb, :],
                                             pt[:, SBW:2 * SBW],
                                             start=(kb == 0),
                                             stop=(kb == NKB - 1))
                        # normalize: ctxT[d, q] = ctx'[d, q] / rowsum[q].
                        # Keep the chain short — the ctx PSUM slots are held
                        # until the multiplies read them. partition_broadcast
                        # is only correct with base partition 0 on both sides.
                        rs = apool.tile([1, 2 * SBW], f32, tag="rs")
                        nc.vector.tensor_copy(rs[:, 0:SBW], pc0[DH:DH + 1, :])
                        nc.vector.tensor_copy(rs[:, SBW:2 * SBW],
                                              pc1[DH:DH + 1, :])
                        rb = [apool.tile([DH, SBW], f32, tag=f"rb{h}",
                                         name=f"rb{h}") for h in range(2)]
                        for h in range(2):
                            nc.gpsimd.partition_broadcast(
                                rb[h][:, :], rs[:, SBW * h:SBW * (h + 1)])
                            nc.vector.reciprocal_approx_fast(rb[h][:, :],
                                                             rb[h][:, :])
                        nc.vector.tensor_mul(ctxT_sb[0:DH, blk, :],
                                             pc0[0:DH, :], rb[0][:, :])
                        nc.vector.tensor_mul(ctxT_sb[DH:2 * DH, blk, :],
                                             pc1[0:DH, :], rb[1][:, :])
                    if dbg and b == 0:
                        for h in range(2):
                            nc.gpsimd.dma_start(dbg_va[h, :, :, :],
                                                va[h][:, :, :])
                    # stage this batch's two A2A blocks as soon as ready
                    for j in (2 * b, 2 * b + 1):
                        nc.sync.dma_start(a2a_in[j, :, :],
                                          ctxT_sb[:, 2 * j:2 * j + 2, :])

                if dbg:
                    nc.gpsimd.dma_start(dbg_qt[:, :, :], qt_sb[:, :, :])
                    nc.gpsimd.dma_start(dbg_kt[:, :, :], kt_sb[:, :, :])
                    nc.gpsimd.dma_start(dbg_ctxT[:, :, :], ctxT_sb[:, :, :])

            # ================= AllToAll: head-split -> seq-split ==========
            nc.gpsimd.collective_compute(
                "AllToAll", ALU.bypass,
                replica_groups=[list(range(N_CORES))],
                ins=[a2a_in[:].opt()], outs=[a2a_out[:].opt()])

            # ============ phases 3+4: out-proj + residual + LayerNorm =====
            with tc.tile_pool(name="outp", bufs=2) as opool:
                ctxF_sb = opool.tile([128, NHC, RS], bf16, tag="ctxF", bufs=1)
                for src in range(N_CORES):
                    nc.sync.dma_start(ctxF_sb[:, src, :], a2a_out[src, :, :])
                if dbg:
                    nc.gpsimd.dma_start(dbg_ctxF[:, :, :], ctxF_sb[:, :, :])

                inv_h = float(1.0 / H)
                for t in range(RS // 128):
                    x_sb = opool.tile([128, H], f32, tag="xln")
                    for g in range(2):
                        po = psA.tile([128, SBW], f32, tag="proj")
                        for c in range(NHC):
                            nc.tensor.matmul(
                                po[:, :],
                                ctxF_sb[:, c, 128 * t:128 * (t + 1)],
                                wo_sb[:, c, SBW * g:SBW * (g + 1)],
                                start=(c == 0), stop=(c == NHC - 1))
                        nc.vector.tensor_add(
                            x_sb[:, SBW * g:SBW * (g + 1)], po[:, :],
                            hres_sb[:, t, SBW * g:SBW * (g + 1)])
                    ssum = opool.tile([128, 1], f32, tag="ssum")
                    nc.vector.tensor_reduce(ssum[:, :], x_sb[:, :],
                                            mybir.AxisListType.X, ALU.add)
                    negmu = opool.tile([128, 1], f32, tag="negmu")
                    nc.vector.tensor_scalar_mul(negmu[:, :], ssum[:, :],
                                                -inv_h)
                    xc = opool.tile([128, H], f32, tag="xc")
                    nc.vector.tensor_scalar_add(xc[:, :], x_sb[:, :],
                                                negmu[:, :])
                    ssq = opool.tile([128, 1], f32, tag="ssq")
                    # x_sb is dead after centering; reuse as Square scratch
                    nc.scalar.activation(x_sb[:, :], xc[:, :], AF.Square,
                                         accum_out=ssq[:, :])
                    var = opool.tile([128, 1], f32, tag="var")
                    nc.vector.tensor_scalar(var[:, :], ssq[:, :], inv_h,
                                            LN_EPS, ALU.mult, ALU.add)
                    rv = opool.tile([128, 1], f32, tag="rv")
                    nc.vector.reciprocal(rv[:, :], var[:, :])
                    rstd = opool.tile([128, 1], f32, tag="rstd")
                    nc.scalar.activation(rstd[:, :], rv[:, :], AF.Sqrt)
                    y_sb = opool.tile([128, H], f32, tag="yln")
                    nc.vector.scalar_tensor_tensor(y_sb[:, :], xc[:, :],
                                                   rstd[:, :], gamma_b[:, :],
                                                   ALU.mult, ALU.mult)
                    nc.vector.tensor_add(y_sb[:, :], y_sb[:, :], beta_b[:, :])
                    nc.sync.dma_start(out_d[128 * t:128 * (t + 1), :],
                                      y_sb[:, :])
            oload.release()

    nc.compile()
    return nc


_NC_CACHE = None


def _get_nc():
    global _NC_CACHE
    if _NC_CACHE is None:
        _NC_CACHE = _build()
    return _NC_CACHE


def _make_in_maps(hidden_states, attention_mask, Wq, bq, Wk, bk, Wv, bv, Wo,
                  bo, ln_gamma, ln_beta):
    hid2 = np.asarray(hidden_states, np.float32).reshape(R, H)
    hT_bf = np.ascontiguousarray(hid2.T).astype(ml_dtypes.bfloat16)
    woT = np.ascontiguousarray(np.asarray(Wo, np.float32).T).astype(
        ml_dtypes.bfloat16)
    maskT = np.ascontiguousarray(
        np.asarray(attention_mask, np.float32).reshape(B, S).T)
    bo32 = np.asarray(bo, np.float32)
    gamma32 = np.asarray(ln_gamma, np.float32)
    beta32 = np.asarray(ln_beta, np.float32)

    in_maps = []
    for c in range(N_CORES):
        sl = slice(HB * c, HB * (c + 1))
        in_maps.append({
            "hT": hT_bf,
            "wqT": np.ascontiguousarray(np.asarray(Wq, np.float32)[sl, :].T
                                        ).astype(ml_dtypes.bfloat16),
            "wkT": np.ascontiguousarray(np.asarray(Wk, np.float32)[sl, :].T
                                        ).astype(ml_dtypes.bfloat16),
            "wvT": np.ascontiguousarray(np.asarray(Wv, np.float32)[sl, :].T
                                        ).astype(ml_dtypes.bfloat16),
            "woT": woT,
            "bq": np.ascontiguousarray(np.asarray(bq, np.float32)[sl]),
            "bk": np.ascontiguousarray(np.asarray(bk, np.float32)[sl]),
            "bv": np.ascontiguousarray(np.asarray(bv, np.float32)[sl]),
            "bo": bo32,
            "gamma": gamma32,
            "beta": beta32,
            "maskT": maskT,
            "hres": np.ascontiguousarray(hid2[RS * c:RS * (c + 1), :]),
        })
    return in_maps


def kernel(hidden_states, attention_mask, Wq, bq, Wk, bk, Wv, bv, Wo, bo,
           ln_gamma, ln_beta):
    global last_exec_time_ns
    from concourse.bass_utils import run_bass_kernel_spmd

    _install_ntff_shim()
    in_maps = _make_in_maps(hidden_states, attention_mask, Wq, bq, Wk, bk,
                            Wv, bv, Wo, bo, ln_gamma, ln_beta)
    nc = _get_nc()
    trace = os.environ.get("BASS_KERNEL_TRACE", "0") == "1"
    res = run_bass_kernel_spmd(nc, in_maps, core_ids=list(range(N_CORES)),
                               trace=trace)
    last_exec_time_ns = res.exec_time_ns
    if trace and res.exec_time_ns is not None:
        print(f"HW exec time: {res.exec_time_ns} ns")

    out = np.concatenate([res.results[c]["out"] for c in range(N_CORES)],
                         axis=0)
    return out.reshape(B, S, H).astype(np.float32)



# revision 8
# speedup vs baseline: 1.1841x; 1.1841x over previous
"""BertAttention (QKV proj + MHA + out-proj + residual + LayerNorm) on 8
Trainium2 NeuronCores -- fully local, zero-collective version.

Sharding: each core owns a 1024-row shard of the flattened (B*S, H)
output: core c -> batch b=c//2, seq half c%2.  The core computes K/V
projections for its WHOLE batch (2048 keys, all 16 heads; K/V proj is
duplicated across the 2 cores of a batch -- cheaper than the AllToAll it
replaces), Q projection for its own 1024 rows, attention for all 16
heads over its rows, then output projection + residual + LayerNorm with
a fully local contraction.  No cross-device traffic at all.

Precision: fp8(e4m3) + DoubleRow (2 key-blocks / 2 contraction chunks
per matmul) for the K/V/Q/out projections and the probs@V contraction;
bf16 for the score matmuls (contraction DH=64, two heads row-tiled into
the 128-row PE array concurrently); fp32 softmax statistics, residual
accumulate in fp32 from a bf16 residual, LayerNorm fp32.  fp8 operands
with small magnitudes are pre-scaled by 64 on the host (weights) and
rescaled in the PSUM-drain ops; the attention-path error this introduces
is suppressed ~60x in the output by the residual (attention out std
~0.017 vs residual std ~1.0).

softmax: scores are built transposed (scoresT[k, q]) so probs@V needs no
transpose; the row-sum comes from a 65th all-ones column on V scaled by
exp(mask)/64, which makes reciprocal(rowsum) directly produce the x64
scaling that keeps fp8 ctx in the e4m3 normal range.
"""

import os
import sys
import contextlib
import ctypes
import types

import numpy as np
import ml_dtypes

N_CORES = 8
B, S, H = 4, 2048, 1024
NH, DH = 16, 64
R = B * S            # 8192 flattened rows
RS = R // N_CORES    # 1024 rows per core (output shard)
NCH = H // 128       # 8 contraction chunks of 128
NP = NH // 2         # 8 head pairs
NKB = S // 128       # 16 key blocks
NKP = NKB // 2       # 8 key-block pairs
NQG = 2              # query groups per core
QW = RS // NQG       # 512 queries per group
VW = DH + 1          # va width: 64 dims + rowsum ones column
LN_EPS = 1e-12

last_exec_time_ns = None

# ---------------------------------------------------------------------------
# NTFF profile hook shim (axon images without antenv.axon_hooks).
_SO_PATH = "/opt/axon/libaxon_pjrt.so"


def _install_ntff_shim():
    try:
        from antenv import axon_hooks  # noqa: F401
        return
    except ImportError:
        pass
    hook = None
    try:
        lib = ctypes.CDLL(_SO_PATH)
        if hasattr(lib, "axon_start_nrt_profile"):
            lib.axon_start_nrt_profile.argtypes = [
                ctypes.POINTER(ctypes.c_int64), ctypes.c_size_t]
            lib.axon_start_nrt_profile.restype = ctypes.c_int64
            lib.axon_stop_nrt_profile.argtypes = [ctypes.c_char_p]
            lib.axon_stop_nrt_profile.restype = ctypes.c_int64

            @contextlib.contextmanager
            def _hook(output_dir, device_ids):
                import jax
                jax.devices()
                if device_ids:
                    ids = (ctypes.c_int64 * len(device_ids))(*device_ids)
                    rc = lib.axon_start_nrt_profile(ids, len(device_ids))
                else:
                    rc = lib.axon_start_nrt_profile(None, 0)
                if rc != 0:
                    raise RuntimeError(f"axon_start_nrt_profile rc={rc}")
                try:
                    yield
                finally:
                    n = lib.axon_stop_nrt_profile(str(output_dir).encode())
                    print(f"profile: {n} ntff file(s) in {output_dir}",
                          file=sys.stderr)

            hook = _hook
    except OSError:
        pass
    mod = types.ModuleType("antenv.axon_hooks")
    mod._hook = hook
    mod.get_axon_ntff_profile_hook = lambda: mod._hook
    mod.set_axon_ntff_profile_hook = lambda h: setattr(mod, "_hook", h)
    sys.modules["antenv.axon_hooks"] = mod
    try:
        import antenv
        antenv.axon_hooks = mod
    except ImportError:
        pass


# ---------------------------------------------------------------------------

def _build():
    from concourse import bacc, tile
    import concourse.mybir as mybir

    f32 = mybir.dt.float32
    bf16 = mybir.dt.bfloat16
    fp8 = mybir.dt.float8e4
    AF = mybir.ActivationFunctionType
    ALU = mybir.AluOpType
    DR = mybir.MatmulPerfMode.DoubleRow

    nc = bacc.Bacc("TRN2", target_bir_lowering=False, debug=False,
                   num_devices=N_CORES)

    # ---- DRAM I/O (per core; b = batch, rows = this core's 1024) ----
    hT_d = nc.dram_tensor("hT", [H, S], fp8, kind="ExternalInput")
    wq_d = nc.dram_tensor("wq", [H, H], fp8, kind="ExternalInput")
    wk_d = nc.dram_tensor("wk", [H, H], fp8, kind="ExternalInput")
    wv_d = nc.dram_tensor("wv", [H, H], fp8, kind="ExternalInput")
    wo_d = nc.dram_tensor("wo", [H, H], fp8, kind="ExternalInput")
    bq_d = nc.dram_tensor("bq", [128, NP], f32, kind="ExternalInput")
    bk_d = nc.dram_tensor("bk", [128, NP], f32, kind="ExternalInput")
    bv_d = nc.dram_tensor("bv", [H], f32, kind="ExternalInput")
    gamma_d = nc.dram_tensor("gamma", [H], f32, kind="ExternalInput")
    beta_d = nc.dram_tensor("beta", [H], f32, kind="ExternalInput")
    maskT_d = nc.dram_tensor("maskT", [128, NKB], f32, kind="ExternalInput")
    hres_d = nc.dram_tensor("hres", [RS, H], bf16, kind="ExternalInput")
    out_d = nc.dram_tensor("out", [RS, H], f32, kind="ExternalOutput")

    with tile.TileContext(nc) as tc:
        with (
            tc.tile_pool(name="const", bufs=1) as cpool,
            tc.tile_pool(name="psP", bufs=2, space="PSUM") as psP,
            tc.tile_pool(name="psS", bufs=2, space="PSUM") as psS,
            tc.tile_pool(name="psC", bufs=1, space="PSUM") as psC,
        ):
            # ================= persistent SBUF =================
            # Early-needed weights first so their DMAs clear the queue
            # before anything compute-blocking.
            wk_sb = cpool.tile([128, NCH, H], fp8, tag="wk")
            wv_sb = cpool.tile([128, NCH, H], fp8, tag="wv")
            wq_sb = cpool.tile([128, NCH, H], fp8, tag="wq")
            hT_sb = cpool.tile([128, NCH, S], fp8, tag="hT")
            for c in range(NCH):
                nc.sync.dma_start(wk_sb[:, c, :], wk_d[128 * c:128 * (c + 1), :])
            # first half of the keys for every chunk, so K/V proj of the
            # first key blocks can start before hT fully lands
            for c in range(NCH):
                nc.sync.dma_start(hT_sb[:, c, 0:1024],
                                  hT_d[128 * c:128 * (c + 1), 0:1024])
            for c in range(NCH):
                nc.sync.dma_start(wv_sb[:, c, :], wv_d[128 * c:128 * (c + 1), :])
                nc.sync.dma_start(wq_sb[:, c, :], wq_d[128 * c:128 * (c + 1), :])
            for c in range(NCH):
                nc.sync.dma_start(hT_sb[:, c, 1024:2048],
                                  hT_d[128 * c:128 * (c + 1), 1024:2048])

            bq_sb = cpool.tile([128, NP], f32, tag="bq")
            bk_sb = cpool.tile([128, NP], f32, tag="bk")
            nc.sync.dma_start(bq_sb[:, :], bq_d[:, :])
            nc.sync.dma_start(bk_sb[:, :], bk_d[:, :])
            bv_b = cpool.tile([128, H], f32, tag="bv_b")
            nc.sync.dma_start(bv_b[:, :],
                              bv_d[:].unsqueeze(0).partition_broadcast(128))

            # exp(mask) per (key-in-block, kblock); em64 = em/64 feeds the
            # rowsum ones-column so 1/rowsum lands pre-scaled by 64.
            em_sb = cpool.tile([128, NKB], f32, tag="em")
            em64_sb = cpool.tile([128, NKB], f32, tag="em64")
            zero_h = cpool.tile([128, NH], f32, tag="zero_h")
            nc.sync.dma_start(em_sb[:, :], maskT_d[:, :])
            nc.scalar.activation(em_sb[:, :], em_sb[:, :], AF.Exp)
            nc.vector.tensor_scalar_mul(em64_sb[:, :], em_sb[:, :],
                                        float(1.0 / 64.0))
            nc.vector.memset(zero_h[:, :], 0.0)

            # big persistent activations
            kt_sb = cpool.tile([128, NP, S], bf16, tag="kt")
            qt_sb = cpool.tile([128, NP, RS], bf16, tag="qt")
            va_sb = cpool.tile([128, NKP, 2, NH, VW], fp8, tag="va")
            ctx_sb = cpool.tile([128, NCH, RS], fp8, tag="ctx")

            # out-proj phase params (DMAs overlap the early compute)
            wo_sb = cpool.tile([128, NCH, H], fp8, tag="wo")
            hres_sb = cpool.tile([128, RS // 128, H], bf16, tag="hres")
            gamma_b = cpool.tile([128, H], f32, tag="gamma_b")
            beta_b = cpool.tile([128, H], f32, tag="beta_b")
            for c in range(NCH):
                nc.sync.dma_start(wo_sb[:, c, :], wo_d[128 * c:128 * (c + 1), :])
            # hres already carries +bo (folded on host)
            for t in range(RS // 128):
                nc.sync.dma_start(hres_sb[:, t, :],
                                  hres_d[128 * t:128 * (t + 1), :])
            nc.sync.dma_start(
                gamma_b[:, :], gamma_d[:].unsqueeze(0).partition_broadcast(128))
            nc.sync.dma_start(
                beta_b[:, :], beta_d[:].unsqueeze(0).partition_broadcast(128))

            with (
                tc.tile_pool(name="ptp", bufs=3) as ptpool,
                tc.tile_pool(name="work", bufs=2) as wpool,
            ):
                # ============ K/V projections (full batch, all heads) =====
                # kt[d-pair, keys]: partitions 0:64 head 2p dims, 64:128
                # head 2p+1; accumulated over 4 DoubleRow chunk-pairs.
                for kbp in range(NKP):
                    k0 = 256 * kbp
                    for p in range(NP):
                        pk = psP.tile([128, 512], f32, tag="proj")
                        for i in range(4):
                            nc.tensor.matmul(
                                pk[:, 0:256],
                                wk_sb[:, 2 * i:2 * i + 2, 128 * p:128 * (p + 1)],
                                hT_sb[:, 2 * i:2 * i + 2, k0:k0 + 256],
                                start=(i == 0), stop=(i == 3), perf_mode=DR)
                        nc.vector.tensor_scalar(
                            kt_sb[:, p, k0:k0 + 256], pk[:, 0:256],
                            bk_sb[:, p:p + 1], float(1.0 / 64.0),
                            ALU.add, ALU.mult)
                    # va natural [keys, d] with ones column; mask folded in
                    for kb in (2 * kbp, 2 * kbp + 1):
                        sl = kb % 2
                        emcol = em_sb[:, kb:kb + 1]
                        for g in range(2):  # d halves: heads 8g..8g+7
                            pv = psP.tile([128, 512], f32, tag="proj")
                            for i in range(4):
                                nc.tensor.matmul(
                                    pv[:, :],
                                    hT_sb[:, 2 * i:2 * i + 2,
                                          128 * kb:128 * (kb + 1)],
                                    wv_sb[:, 2 * i:2 * i + 2,
                                          512 * g:512 * (g + 1)],
                                    start=(i == 0), stop=(i == 3), perf_mode=DR)
                            t1 = wpool.tile([128, 512], f32, tag="t1")
                            nc.vector.scalar_tensor_tensor(
                                t1[:, :], pv[:, :], float(1.0 / 64.0),
                                bv_b[:, 512 * g:512 * (g + 1)],
                                ALU.mult, ALU.add)
                            nc.vector.tensor_scalar_mul(
                                va_sb[:, kbp, sl, 8 * g:8 * (g + 1), 0:DH],
                                t1[:, :], emcol)
                        # ones column (= exp(mask)/64) for the rowsum row
                        nc.vector.tensor_scalar_add(
                            va_sb[:, kbp, sl, :, DH:DH + 1],
                            zero_h[:, :].unsqueeze(2),
                            em64_sb[:, kb:kb + 1])

                # ============ Q projection (own 1024 rows) ================
                q0 = 0  # hT columns of this core's rows are sent via hresQ
                for p in range(NP):
                    for qg in range(NQG):
                        pq = psP.tile([128, 512], f32, tag="proj")
                        for i in range(4):
                            nc.tensor.matmul(
                                pq[:, :],
                                wq_sb[:, 2 * i:2 * i + 2, 128 * p:128 * (p + 1)],
                                hTq_view(nc, hT_sb, i, qg),
                                start=(i == 0), stop=(i == 3), perf_mode=DR)
                        nc.vector.tensor_scalar(
                            qt_sb[:, p, QW * qg:QW * (qg + 1)], pq[:, :],
                            bq_sb[:, p:p + 1], float(1.0 / 64.0),
                            ALU.add, ALU.mult)

                # ============ attention + out-proj, per query group =======
                for qg in range(NQG):
                    for p in range(NP):
                        cps = psC.tile([128, 1024], f32, tag="ctxp")
                        for kbp in range(NKP):
                            pt = ptpool.tile([128, 2, 2, 512], fp8, tag="pt")
                            for sl in range(2):
                                kb = 2 * kbp + sl
                                sc = psS.tile([128, 2, 512], f32, tag="sc")
                                nc.tensor.matmul(
                                    sc[:, 0, :],
                                    kt_sb[0:DH, p, 128 * kb:128 * (kb + 1)],
                                    qt_sb[0:DH, p, QW * qg:QW * (qg + 1)],
                                    start=True, stop=True)
                                nc.tensor.matmul(
                                    sc[:, 1, :],
                                    kt_sb[DH:128, p, 128 * kb:128 * (kb + 1)],
                                    qt_sb[DH:128, p, QW * qg:QW * (qg + 1)],
                                    start=True, stop=True)
                                nc.scalar.activation(
                                    pt[:, sl, :, :], sc[:, :, :],
                                    AF.Exp, scale=0.125)
                            for h in range(2):
                                nc.tensor.matmul(
                                    cps[0:VW, 512 * h:512 * h + 512],
                                    va_sb[:, kbp, :, 2 * p + h, :],
                                    pt[:, :, h, :],
                                    start=(kbp == 0), stop=(kbp == NKP - 1),
                                    perf_mode=DR)
                        # normalize: ctx64[d, q] = ctx[d, q] * (64 / rowsum)
                        rs = wpool.tile([1, 1024], f32, tag="rs")
                        nc.vector.tensor_copy(rs[:, :], cps[DH:DH + 1, :])
                        rb = wpool.tile([DH, 1024], f32, tag="rb")
                        nc.gpsimd.partition_broadcast(rb[:, 0:512],
                                                      rs[:, 0:512])
                        nc.gpsimd.partition_broadcast(rb[:, 512:1024],
                                                      rs[:, 512:1024])
                        nc.vector.reciprocal_approx_fast(rb[:, :], rb[:, :])
                        nc.vector.tensor_mul(
                            ctx_sb[0:DH, p, QW * qg:QW * (qg + 1)],
                            cps[0:DH, 0:512], rb[:, 0:512])
                        nc.vector.tensor_mul(
                            ctx_sb[DH:128, p, QW * qg:QW * (qg + 1)],
                            cps[0:DH, 512:1024], rb[:, 512:1024])

                    # ---- out-proj + residual + LayerNorm for this qg ----
                    inv_h = float(1.0 / H)
                    for tt in range(QW // 128):
                        t = (RS // 128 // NQG) * qg + tt
                        x_sb = wpool.tile([128, H], f32, tag="xln")
                        for g in range(2):
                            po = psP.tile([128, 512], f32, tag="proj")
                            for i in range(4):
                                nc.tensor.matmul(
                                    po[:, :],
                                    ctx_sb[:, 2 * i:2 * i + 2,
                                           128 * t:128 * (t + 1)],
                                    wo_sb[:, 2 * i:2 * i + 2,
                                          512 * g:512 * (g + 1)],
                                    start=(i == 0), stop=(i == 3),
                                    perf_mode=DR)
                            nc.vector.scalar_tensor_tensor(
                                x_sb[:, 512 * g:512 * (g + 1)], po[:, :],
                                float(1.0 / 4096.0),
                                hres_sb[:, t, 512 * g:512 * (g + 1)],
                                ALU.mult, ALU.add)
                        ssum = wpool.tile([128, 1], f32, tag="ssum")
                        nc.vector.tensor_reduce(ssum[:, :], x_sb[:, :],
                                                mybir.AxisListType.X, ALU.add)
                        negmu = wpool.tile([128, 1], f32, tag="negmu")
                        nc.vector.tensor_scalar_mul(negmu[:, :], ssum[:, :],
                                                    -inv_h)
                        xc = wpool.tile([128, H], f32, tag="xc")
                        nc.vector.tensor_scalar_add(xc[:, :], x_sb[:, :],
                                                    negmu[:, :])
                        ssq = wpool.tile([128, 1], f32, tag="ssq")
                        # x_sb is dead after centering; reuse as xc^2 scratch
                        nc.vector.scalar_tensor_tensor(
                            x_sb[:, :], xc[:, :], 1.0, xc[:, :],
                            ALU.mult, ALU.mult, accum_out=ssq[:, :])
                        var = wpool.tile([128, 1], f32, tag="var")
                        nc.vector.tensor_scalar(var[:, :], ssq[:, :], inv_h,
                                                LN_EPS, ALU.mult, ALU.add)
                        rv = wpool.tile([128, 1], f32, tag="rv")
                        nc.vector.reciprocal(rv[:, :], var[:, :])
                        rstd = wpool.tile([128, 1], f32, tag="rstd")
                        nc.scalar.activation(rstd[:, :], rv[:, :], AF.Sqrt)
                        y_sb = wpool.tile([128, H], f32, tag="yln")
                        nc.vector.scalar_tensor_tensor(
                            y_sb[:, :], xc[:, :], rstd[:, :], gamma_b[:, :],
                            ALU.mult, ALU.mult)
                        nc.vector.tensor_add(y_sb[:, :], y_sb[:, :],
                                             beta_b[:, :])
                        nc.sync.dma_start(out_d[128 * t:128 * (t + 1), :],
                                          y_sb[:, :])

    nc.compile()
    return nc


def hTq_view(nc, hT_sb, i, qg):
    """hT columns of this core's own rows: handled by the host passing the
    core's row block at hT columns [0, RS) of a separate region?  No --
    the core's rows ARE a column slice of its batch hT: half = core % 2,
    columns 1024*half + [0, 1024).  The host rolls hT so that this core's
    own rows always sit in columns [0, 1024).  See _make_in_maps."""
    return hT_sb[:, 2 * i:2 * i + 2, QW * qg:QW * (qg + 1)]


_NC_CACHE = None


def _get_nc():
    global _NC_CACHE
    if _NC_CACHE is None:
        _NC_CACHE = _build()
    return _NC_CACHE


def _make_in_maps(hidden_states, attention_mask, Wq, bq, Wk, bk, Wv, bv, Wo,
                  bo, ln_gamma, ln_beta):
    f8 = ml_dtypes.float8_e4m3fn
    hid2 = np.asarray(hidden_states, np.float32).reshape(R, H)
    wq8 = np.ascontiguousarray(np.asarray(Wq, np.float32).T * 64.0).astype(f8)
    wk8 = np.ascontiguousarray(np.asarray(Wk, np.float32).T * 64.0).astype(f8)
    wv8 = np.ascontiguousarray(np.asarray(Wv, np.float32).T * 64.0).astype(f8)
    wo8 = np.ascontiguousarray(np.asarray(Wo, np.float32).T * 64.0).astype(f8)
    bq2 = np.ascontiguousarray(
        (np.asarray(bq, np.float32) * 64.0).reshape(NP, 128).T)
    bk2 = np.ascontiguousarray(
        (np.asarray(bk, np.float32) * 64.0).reshape(NP, 128).T)
    bv32 = np.asarray(bv, np.float32)
    bo32 = np.asarray(bo, np.float32)
    gamma32 = np.asarray(ln_gamma, np.float32)
    beta32 = np.asarray(ln_beta, np.float32)
    mask2 = np.asarray(attention_mask, np.float32).reshape(B, S)

    in_maps = []
    for c in range(N_CORES):
        b, half = divmod(c, 2)
        hb = hid2[S * b:S * (b + 1), :]          # this batch's rows [2048, H]
        # roll so this core's own 1024 rows sit in columns [0, 1024) of hT
        hb_roll = np.roll(hb, -RS * half, axis=0)
        hT8 = np.ascontiguousarray(hb_roll.T).astype(f8)
        # mask key order must match the rolled key order
        m_roll = np.roll(mask2[b], -RS * half)
        maskT = np.ascontiguousarray(m_roll.reshape(NKB, 128).T)
        in_maps.append({
            "hT": hT8,
            "wq": wq8, "wk": wk8, "wv": wv8, "wo": wo8,
            "bq": bq2, "bk": bk2, "bv": bv32,
            "gamma": gamma32, "beta": beta32,
            "maskT": maskT,
            "hres": np.ascontiguousarray(
                hid2[RS * c:RS * (c + 1), :] + bo32[None, :]
            ).astype(ml_dtypes.bfloat16),
        })
    return in_maps


def kernel(hidden_states, attention_mask, Wq, bq, Wk, bk, Wv, bv, Wo, bo,
           ln_gamma, ln_beta):
    global last_exec_time_ns
    from concourse.bass_utils import run_bass_kernel_spmd

    _install_ntff_shim()
    in_maps = _make_in_maps(hidden_states, attention_mask, Wq, bq, Wk, bk,
                            Wv, bv, Wo, bo, ln_gamma, ln_beta)
    nc = _get_nc()
    trace = os.environ.get("BASS_KERNEL_TRACE", "0") == "1"
    res = run_bass_kernel_spmd(nc, in_maps, core_ids=list(range(N_CORES)),
                               trace=trace)
    last_exec_time_ns = res.exec_time_ns
    if trace and res.exec_time_ns is not None:
        print(f"HW exec time: {res.exec_time_ns} ns")

    out = np.concatenate([res.results[c]["out"] for c in range(N_CORES)],
                         axis=0)
    return out.reshape(B, S, H).astype(np.float32)


# revision 14
# speedup vs baseline: 1.4748x; 1.2455x over previous
"""BertAttention (QKV proj + MHA + out-proj + residual + LayerNorm) on 8
Trainium2 NeuronCores -- fully local, zero-collective version.

Sharding: each core owns a 1024-row shard of the flattened (B*S, H)
output: core c -> batch b=c//2, seq half c%2.  The core computes K/V
projections for its WHOLE batch (2048 keys, all 16 heads; K/V proj is
duplicated across the 2 cores of a batch -- cheaper than the AllToAll it
replaces), Q projection for its own 1024 rows, attention for all 16
heads over its rows, then output projection + residual + LayerNorm with
a fully local contraction.  No cross-device traffic at all.

Precision: fp8(e4m3) + DoubleRow (2 key-blocks / 2 contraction chunks
per matmul) for the K/V/Q/out projections and the probs@V contraction;
bf16 for the score matmuls (contraction DH=64, two heads row-tiled into
the 128-row PE array concurrently); fp32 softmax statistics, residual
accumulate in fp32 from a bf16 residual, LayerNorm fp32.  fp8 operands
with small magnitudes are pre-scaled by 64 on the host (weights) and
rescaled in the PSUM-drain ops; the attention-path error this introduces
is suppressed ~60x in the output by the residual (attention out std
~0.017 vs residual std ~1.0).

softmax: scores are built transposed (scoresT[k, q]) so probs@V needs no
transpose; the row-sum comes from a 65th all-ones column on V scaled by
exp(mask)/64, which makes reciprocal(rowsum) directly produce the x64
scaling that keeps fp8 ctx in the e4m3 normal range.
"""

import os
import sys
import contextlib
import ctypes
import types

import numpy as np
import ml_dtypes

N_CORES = 8
B, S, H = 4, 2048, 1024
NH, DH = 16, 64
R = B * S            # 8192 flattened rows
RS = R // N_CORES    # 1024 rows per core (output shard)
NCH = H // 128       # 8 contraction chunks of 128
NP = NH // 2         # 8 head pairs
NKB = S // 128       # 16 key blocks
NKP = NKB // 2       # 8 key-block pairs
NQG = 2              # query groups per core
QW = RS // NQG       # 512 queries per group
VW = DH + 1          # va width: 64 dims + rowsum ones column
LN_EPS = 1e-12

last_exec_time_ns = None

# ---------------------------------------------------------------------------
# NTFF profile hook shim (axon images without antenv.axon_hooks).
_SO_PATH = "/opt/axon/libaxon_pjrt.so"


def _install_ntff_shim():
    try:
        from antenv import axon_hooks  # noqa: F401
        return
    except ImportError:
        pass
    hook = None
    try:
        lib = ctypes.CDLL(_SO_PATH)
        if hasattr(lib, "axon_start_nrt_profile"):
            lib.axon_start_nrt_profile.argtypes = [
                ctypes.POINTER(ctypes.c_int64), ctypes.c_size_t]
            lib.axon_start_nrt_profile.restype = ctypes.c_int64
            lib.axon_stop_nrt_profile.argtypes = [ctypes.c_char_p]
            lib.axon_stop_nrt_profile.restype = ctypes.c_int64

            @contextlib.contextmanager
            def _hook(output_dir, device_ids):
                import jax
                jax.devices()
                if device_ids:
                    ids = (ctypes.c_int64 * len(device_ids))(*device_ids)
                    rc = lib.axon_start_nrt_profile(ids, len(device_ids))
                else:
                    rc = lib.axon_start_nrt_profile(None, 0)
                if rc != 0:
                    raise RuntimeError(f"axon_start_nrt_profile rc={rc}")
                try:
                    yield
                finally:
                    n = lib.axon_stop_nrt_profile(str(output_dir).encode())
                    print(f"profile: {n} ntff file(s) in {output_dir}",
                          file=sys.stderr)

            hook = _hook
    except OSError:
        pass
    mod = types.ModuleType("antenv.axon_hooks")
    mod._hook = hook
    mod.get_axon_ntff_profile_hook = lambda: mod._hook
    mod.set_axon_ntff_profile_hook = lambda h: setattr(mod, "_hook", h)
    sys.modules["antenv.axon_hooks"] = mod
    try:
        import antenv
        antenv.axon_hooks = mod
    except ImportError:
        pass


# ---------------------------------------------------------------------------

def _build():
    from concourse import bacc, tile
    import concourse.mybir as mybir

    f32 = mybir.dt.float32
    bf16 = mybir.dt.bfloat16
    fp8 = mybir.dt.float8e4
    AF = mybir.ActivationFunctionType
    ALU = mybir.AluOpType
    DR = mybir.MatmulPerfMode.DoubleRow

    nc = bacc.Bacc("TRN2", target_bir_lowering=False, debug=False,
                   num_devices=N_CORES)

    # ---- DRAM I/O (per core; b = batch, rows = this core's 1024) ----
    hT_d = nc.dram_tensor("hT", [H, S], fp8, kind="ExternalInput")
    wq_d = nc.dram_tensor("wq", [H, H], fp8, kind="ExternalInput")
    wk_d = nc.dram_tensor("wk", [H, H], fp8, kind="ExternalInput")
    wv_d = nc.dram_tensor("wv", [H, H], fp8, kind="ExternalInput")
    wo_d = nc.dram_tensor("wo", [H, H], fp8, kind="ExternalInput")
    bq_d = nc.dram_tensor("bq", [128, NP], f32, kind="ExternalInput")
    bk_d = nc.dram_tensor("bk", [128, NP], f32, kind="ExternalInput")
    bv_d = nc.dram_tensor("bv", [H], f32, kind="ExternalInput")
    gamma_d = nc.dram_tensor("gamma", [H], f32, kind="ExternalInput")
    beta_d = nc.dram_tensor("beta", [H], f32, kind="ExternalInput")
    maskT_d = nc.dram_tensor("maskT", [128, NKB], f32, kind="ExternalInput")
    hres_d = nc.dram_tensor("hres", [RS, H], bf16, kind="ExternalInput")
    out_d = nc.dram_tensor("out", [RS, H], f32, kind="ExternalOutput")

    with tile.TileContext(nc) as tc:
        with (
            tc.tile_pool(name="const", bufs=1) as cpool,
            tc.tile_pool(name="psP", bufs=2, space="PSUM") as psP,
            tc.tile_pool(name="psS", bufs=2, space="PSUM") as psS,
            tc.tile_pool(name="psC", bufs=1, space="PSUM") as psC,
        ):
            # ================= persistent SBUF =================
            # Early-needed weights first so their DMAs clear the queue
            # before anything compute-blocking.
            wk_sb = cpool.tile([128, NCH, H], fp8, tag="wk")
            wv_sb = cpool.tile([128, NCH, H], fp8, tag="wv")
            wq_sb = cpool.tile([128, NCH, H], fp8, tag="wq")
            hT_sb = cpool.tile([128, NCH, S], fp8, tag="hT")
            for c in range(NCH):
                nc.sync.dma_start(wk_sb[:, c, :], wk_d[128 * c:128 * (c + 1), :])
            # first half of the keys for every chunk, so K/V proj of the
            # first key blocks can start before hT fully lands
            for c in range(NCH):
                nc.sync.dma_start(hT_sb[:, c, 0:1024],
                                  hT_d[128 * c:128 * (c + 1), 0:1024])
            for c in range(NCH):
                nc.sync.dma_start(wv_sb[:, c, :], wv_d[128 * c:128 * (c + 1), :])
                nc.sync.dma_start(wq_sb[:, c, :], wq_d[128 * c:128 * (c + 1), :])
            for c in range(NCH):
                nc.sync.dma_start(hT_sb[:, c, 1024:2048],
                                  hT_d[128 * c:128 * (c + 1), 1024:2048])

            bq_sb = cpool.tile([128, NP], f32, tag="bq")
            bk_sb = cpool.tile([128, NP], f32, tag="bk")
            nc.sync.dma_start(bq_sb[:, :], bq_d[:, :])
            nc.sync.dma_start(bk_sb[:, :], bk_d[:, :])
            bv_b = cpool.tile([128, H], f32, tag="bv_b")
            nc.sync.dma_start(bv_b[:, :],
                              bv_d[:].unsqueeze(0).partition_broadcast(128))

            # exp(mask) per (key-in-block, kblock); em64 = em/64 feeds the
            # rowsum ones-column so 1/rowsum lands pre-scaled by 64.
            em_sb = cpool.tile([128, NKB], f32, tag="em")
            em64_sb = cpool.tile([128, NKB], f32, tag="em64")
            zero_h = cpool.tile([128, NH], f32, tag="zero_h")
            nc.sync.dma_start(em_sb[:, :], maskT_d[:, :])
            nc.scalar.activation(em_sb[:, :], em_sb[:, :], AF.Exp)
            nc.vector.tensor_scalar_mul(em64_sb[:, :], em_sb[:, :],
                                        float(1.0 / 64.0))
            nc.vector.memset(zero_h[:, :], 0.0)

            # big persistent activations
            kt_sb = cpool.tile([128, NP, S], bf16, tag="kt")
            qt_sb = cpool.tile([128, NP, RS], bf16, tag="qt")
            va_sb = cpool.tile([128, NKP, 2, NH, VW], fp8, tag="va")
            ctx_sb = cpool.tile([128, NCH, RS], fp8, tag="ctx")

            # out-proj phase params (DMAs overlap the early compute)
            wo_sb = cpool.tile([128, NCH, H], fp8, tag="wo")
            hres_sb = cpool.tile([128, RS // 128, H], bf16, tag="hres")
            gamma_b = cpool.tile([128, H], f32, tag="gamma_b")
            beta_b = cpool.tile([128, H], f32, tag="beta_b")
            for c in range(NCH):
                nc.sync.dma_start(wo_sb[:, c, :], wo_d[128 * c:128 * (c + 1), :])
            # hres already carries +bo (folded on host)
            for t in range(RS // 128):
                nc.sync.dma_start(hres_sb[:, t, :],
                                  hres_d[128 * t:128 * (t + 1), :])
            nc.sync.dma_start(
                gamma_b[:, :], gamma_d[:].unsqueeze(0).partition_broadcast(128))
            nc.sync.dma_start(
                beta_b[:, :], beta_d[:].unsqueeze(0).partition_broadcast(128))

            with (
                tc.tile_pool(name="ptp", bufs=4) as ptpool,
                tc.tile_pool(name="work", bufs=2) as wpool,
            ):
                # ---------- projection emitters (JIT, woven into the
                # attention stream so ScalarE never starves) ----------
                def k_proj(p, kq):  # 512-key block kq of pair p
                    k0 = 512 * kq
                    pk = psP.tile([128, 512], f32, tag="proj")
                    for i in range(4):
                        nc.tensor.matmul(
                            pk[:, :],
                            wk_sb[:, 2 * i:2 * i + 2, 128 * p:128 * (p + 1)],
                            hT_sb[:, 2 * i:2 * i + 2, k0:k0 + 512],
                            start=(i == 0), stop=(i == 3), perf_mode=DR)
                    nc.vector.tensor_scalar(
                        kt_sb[:, p, k0:k0 + 512], pk[:, :],
                        bk_sb[:, p:p + 1], float(1.0 / 64.0),
                        ALU.add, ALU.mult)

                def q_proj(p, qg):
                    pq = psP.tile([128, 512], f32, tag="proj")
                    for i in range(4):
                        nc.tensor.matmul(
                            pq[:, :],
                            wq_sb[:, 2 * i:2 * i + 2, 128 * p:128 * (p + 1)],
                            hT_sb[:, 2 * i:2 * i + 2, QW * qg:QW * (qg + 1)],
                            start=(i == 0), stop=(i == 3), perf_mode=DR)
                    nc.vector.tensor_scalar(
                        qt_sb[:, p, QW * qg:QW * (qg + 1)], pq[:, :],
                        bq_sb[:, p:p + 1], float(1.0 / 64.0),
                        ALU.add, ALU.mult)

                def v_proj(kb, g):  # heads 8g..8g+7 of key block kb
                    kbp, sl = divmod(kb, 2)
                    emcol = em_sb[:, kb:kb + 1]
                    pv = psP.tile([128, 512], f32, tag="proj")
                    for i in range(4):
                        nc.tensor.matmul(
                            pv[:, :],
                            hT_sb[:, 2 * i:2 * i + 2,
                                  128 * kb:128 * (kb + 1)],
                            wv_sb[:, 2 * i:2 * i + 2,
                                  512 * g:512 * (g + 1)],
                            start=(i == 0), stop=(i == 3), perf_mode=DR)
                    t1 = wpool.tile([128, 512], f32, tag="t1")
                    nc.vector.scalar_tensor_tensor(
                        t1[:, :], pv[:, :], float(1.0 / 64.0),
                        bv_b[:, 512 * g:512 * (g + 1)], ALU.mult, ALU.add)
                    nc.vector.tensor_scalar_mul(
                        va_sb[:, kbp, sl, 8 * g:8 * (g + 1), 0:DH],
                        t1[:, :], emcol)
                    if g == 0:  # rowsum ones column (= exp(mask)/64)
                        nc.vector.tensor_scalar_add(
                            va_sb[:, kbp, sl, :, DH:DH + 1],
                            zero_h[:, :].unsqueeze(2),
                            em64_sb[:, kb:kb + 1])

                def normalize(qg, p, cps):
                    # ctx64[d, q] = ctx[d, q] * (64 / rowsum)
                    rs = wpool.tile([1, 1024], f32, tag="rs")
                    nc.vector.tensor_copy(rs[:, :], cps[DH:DH + 1, :])
                    rb = wpool.tile([DH, 1024], f32, tag="rb")
                    nc.gpsimd.partition_broadcast(rb[:, 0:512], rs[:, 0:512])
                    nc.gpsimd.partition_broadcast(rb[:, 512:1024],
                                                  rs[:, 512:1024])
                    nc.vector.reciprocal_approx_fast(rb[:, :], rb[:, :])
                    nc.vector.tensor_mul(
                        ctx_sb[0:DH, p, QW * qg:QW * (qg + 1)],
                        cps[0:DH, 0:512], rb[:, 0:512])
                    nc.vector.tensor_mul(
                        ctx_sb[DH:128, p, QW * qg:QW * (qg + 1)],
                        cps[0:DH, 512:1024], rb[:, 512:1024])

                inv_h = float(1.0 / H)

                def out_proj_steps(qg):
                    # out-proj + residual + LN for 4 row tiles, as a list of
                    # small closures spread through the next qg's stream so
                    # the PE burst never starves ScalarE.  The 1/sqrt(var)
                    # is batched per qg to avoid ACT exp<->sqrt table swaps
                    # in the middle of the softmax stream.
                    nt = QW // 128
                    xcs = []
                    rv4 = wpool.tile([128, nt], f32, tag="rv4")
                    rstd4 = wpool.tile([128, nt], f32, tag="rstd4")

                    def tile_step(tt):
                        t = nt * qg + tt
                        x_sb = wpool.tile([128, H], f32, tag="xln")
                        for g in range(2):
                            po = psP.tile([128, 512], f32, tag="proj")
                            for i in range(4):
                                nc.tensor.matmul(
                                    po[:, :],
                                    ctx_sb[:, 2 * i:2 * i + 2,
                                           128 * t:128 * (t + 1)],
                                    wo_sb[:, 2 * i:2 * i + 2,
                                          512 * g:512 * (g + 1)],
                                    start=(i == 0), stop=(i == 3),
                                    perf_mode=DR)
                            nc.vector.scalar_tensor_tensor(
                                x_sb[:, 512 * g:512 * (g + 1)], po[:, :],
                                float(1.0 / 4096.0),
                                hres_sb[:, t, 512 * g:512 * (g + 1)],
                                ALU.mult, ALU.add)
                        ssum = wpool.tile([128, 1], f32, tag="ssum")
                        nc.vector.tensor_reduce(ssum[:, :], x_sb[:, :],
                                                mybir.AxisListType.X, ALU.add)
                        negmu = wpool.tile([128, 1], f32, tag="negmu")
                        nc.vector.tensor_scalar_mul(negmu[:, :], ssum[:, :],
                                                    -inv_h)
                        xc = wpool.tile([128, H], f32, tag="xc", bufs=nt)
                        nc.vector.tensor_scalar_add(xc[:, :], x_sb[:, :],
                                                    negmu[:, :])
                        ssq = wpool.tile([128, 1], f32, tag="ssq")
                        # x_sb is dead after centering; reuse as xc^2 scratch
                        nc.vector.scalar_tensor_tensor(
                            x_sb[:, :], xc[:, :], 1.0, xc[:, :],
                            ALU.mult, ALU.mult, accum_out=ssq[:, :])
                        var = wpool.tile([128, 1], f32, tag="var")
                        nc.vector.tensor_scalar(var[:, :], ssq[:, :], inv_h,
                                                LN_EPS, ALU.mult, ALU.add)
                        nc.vector.reciprocal(rv4[:, tt:tt + 1], var[:, :])
                        xcs.append(xc)

                    def sqrt_step():
                        nc.scalar.activation(rstd4[:, :], rv4[:, :], AF.Sqrt)

                    def fin_step(tt):
                        t = nt * qg + tt
                        y_sb = wpool.tile([128, H], f32, tag="xln")
                        nc.vector.scalar_tensor_tensor(
                            y_sb[:, :], xcs[tt][:, :], rstd4[:, tt:tt + 1],
                            gamma_b[:, :], ALU.mult, ALU.mult)
                        nc.vector.tensor_add(y_sb[:, :], y_sb[:, :],
                                             beta_b[:, :])
                        nc.sync.dma_start(out_d[128 * t:128 * (t + 1), :],
                                          y_sb[:, :])

                    steps = [lambda tt=tt: tile_step(tt) for tt in range(nt)]
                    steps.append(sqrt_step)
                    steps += [lambda tt=tt: fin_step(tt) for tt in range(nt)]
                    return steps

                # ---------- flattened attention stream ----------
                # pv runs one (qg,p,kbp) iteration behind scores/exp so the
                # exp stream never waits at pair boundaries.
                pending_pv = None    # (qg, p, kbp, pt)
                out_steps = []       # deferred out-proj work, one per iter
                cps_holder = [None]

                def flush_pv():
                    nonlocal pending_pv
                    if pending_pv is None:
                        return
                    fqg, fp, fkbp, fpt = pending_pv
                    pending_pv = None
                    if fkbp == 0:
                        cps_holder[0] = psC.tile([128, 1024], f32,
                                                 tag="ctxp", name="cps")
                    cps = cps_holder[0]
                    for h in range(2):
                        nc.tensor.matmul(
                            cps[0:VW, 512 * h:512 * h + 512],
                            va_sb[:, fkbp, :, 2 * fp + h, :],
                            fpt[:, :, h, :],
                            start=(fkbp == 0), stop=(fkbp == NKP - 1),
                            perf_mode=DR)
                    if fkbp == NKP - 1:
                        normalize(fqg, fp, cps)

                for qg in range(NQG):
                    for p in range(NP):
                        if qg == 0:
                            if p > 0:
                                for kq in range(4):
                                    k_proj(p, kq)
                            q_proj(p, 0)
                            q_proj(p, 1)
                        for kbp in range(NKP):
                            if qg == 0 and p == 0:
                                if kbp % 2 == 0:
                                    k_proj(0, kbp // 2)
                                v_proj(2 * kbp, 0)
                                v_proj(2 * kbp + 1, 0)
                            if qg == 0 and p == 4:
                                v_proj(2 * kbp, 1)
                                v_proj(2 * kbp + 1, 1)
                            pt = ptpool.tile([128, 2, 2, 512], fp8, tag="pt")
                            for sl in range(2):
                                kb = 2 * kbp + sl
                                sc = psS.tile([128, 2, 512], f32, tag="sc")
                                nc.tensor.matmul(
                                    sc[:, 0, :],
                                    kt_sb[0:DH, p, 128 * kb:128 * (kb + 1)],
                                    qt_sb[0:DH, p, QW * qg:QW * (qg + 1)],
                                    start=True, stop=True)
                                nc.tensor.matmul(
                                    sc[:, 1, :],
                                    kt_sb[DH:128, p, 128 * kb:128 * (kb + 1)],
                                    qt_sb[DH:128, p, QW * qg:QW * (qg + 1)],
                                    start=True, stop=True)
                                nc.scalar.activation(
                                    pt[:, sl, :, :], sc[:, :, :],
                                    AF.Exp, scale=0.125)
                            flush_pv()
                            pending_pv = (qg, p, kbp, pt)
                            if (out_steps and kbp % 4 == 2
                                    and not (qg == 0 and p in (0, 4))):
                                out_steps.pop(0)()
                    # defer this qg's out-proj into the next qg's stream
                    out_steps += out_proj_steps(qg)
                flush_pv()
                for step in out_steps:
                    step()

    nc.compile()
    return nc


_NC_CACHE = None


def _get_nc():
    global _NC_CACHE
    if _NC_CACHE is None:
        _NC_CACHE = _build()
    return _NC_CACHE


def _make_in_maps(hidden_states, attention_mask, Wq, bq, Wk, bk, Wv, bv, Wo,
                  bo, ln_gamma, ln_beta):
    f8 = ml_dtypes.float8_e4m3fn
    hid2 = np.asarray(hidden_states, np.float32).reshape(R, H)
    wq8 = np.ascontiguousarray(np.asarray(Wq, np.float32).T * 64.0).astype(f8)
    wk8 = np.ascontiguousarray(np.asarray(Wk, np.float32).T * 64.0).astype(f8)
    wv8 = np.ascontiguousarray(np.asarray(Wv, np.float32).T * 64.0).astype(f8)
    wo8 = np.ascontiguousarray(np.asarray(Wo, np.float32).T * 64.0).astype(f8)
    bq2 = np.ascontiguousarray(
        (np.asarray(bq, np.float32) * 64.0).reshape(NP, 128).T)
    bk2 = np.ascontiguousarray(
        (np.asarray(bk, np.float32) * 64.0).reshape(NP, 128).T)
    bv32 = np.asarray(bv, np.float32)
    bo32 = np.asarray(bo, np.float32)
    gamma32 = np.asarray(ln_gamma, np.float32)
    beta32 = np.asarray(ln_beta, np.float32)
    mask2 = np.asarray(attention_mask, np.float32).reshape(B, S)

    in_maps = []
    for c in range(N_CORES):
        b, half = divmod(c, 2)
        hb = hid2[S * b:S * (b + 1), :]          # this batch's rows [2048, H]
        # roll so this core's own 1024 rows sit in columns [0, 1024) of hT
        hb_roll = np.roll(hb, -RS * half, axis=0)
        hT8 = np.ascontiguousarray(hb_roll.T).astype(f8)
        # mask key order must match the rolled key order
        m_roll = np.roll(mask2[b], -RS * half)
        maskT = np.ascontiguousarray(m_roll.reshape(NKB, 128).T)
        in_maps.append({
            "hT": hT8,
            "wq": wq8, "wk": wk8, "wv": wv8, "wo": wo8,
            "bq": bq2, "bk": bk2, "bv": bv32,
            "gamma": gamma32, "beta": beta32,
            "maskT": maskT,
            "hres": np.ascontiguousarray(
                hid2[RS * c:RS * (c + 1), :] + bo32[None, :]
            ).astype(ml_dtypes.bfloat16),
        })
    return in_maps


def kernel(hidden_states, attention_mask, Wq, bq, Wk, bk, Wv, bv, Wo, bo,
           ln_gamma, ln_beta):
    global last_exec_time_ns
    from concourse.bass_utils import run_bass_kernel_spmd

    _install_ntff_shim()
    in_maps = _make_in_maps(hidden_states, attention_mask, Wq, bq, Wk, bk,
                            Wv, bv, Wo, bo, ln_gamma, ln_beta)
    nc = _get_nc()
    trace = os.environ.get("BASS_KERNEL_TRACE", "0") == "1"
    res = run_bass_kernel_spmd(nc, in_maps, core_ids=list(range(N_CORES)),
                               trace=trace)
    last_exec_time_ns = res.exec_time_ns
    if trace and res.exec_time_ns is not None:
        print(f"HW exec time: {res.exec_time_ns} ns")

    out = np.concatenate([res.results[c]["out"] for c in range(N_CORES)],
                         axis=0)
    return out.reshape(B, S, H).astype(np.float32)


# revision 25
# speedup vs baseline: 1.5521x; 1.0524x over previous
"""BertAttention (QKV proj + MHA + out-proj + residual + LayerNorm) on 8
Trainium2 NeuronCores -- fully local, zero-collective version.

Sharding: each core owns a 1024-row shard of the flattened (B*S, H)
output: core c -> batch b=c//2, seq half c%2.  The core computes K/V
projections for its WHOLE batch (2048 keys, all 16 heads; K/V proj is
duplicated across the 2 cores of a batch -- cheaper than the AllToAll it
replaces), Q projection for its own 1024 rows, attention for all 16
heads over its rows, then output projection + residual + LayerNorm with
a fully local contraction.  No cross-device traffic at all.

Precision: fp8(e4m3) + DoubleRow (2 key-blocks / 2 contraction chunks
per matmul) for the K/V/Q/out projections and the probs@V contraction;
bf16 for the score matmuls (contraction DH=64, two heads row-tiled into
the 128-row PE array concurrently); fp32 softmax statistics, residual
accumulate in fp32 from a bf16 residual, LayerNorm fp32.  fp8 operands
with small magnitudes are pre-scaled by 64 on the host (weights) and
rescaled in the PSUM-drain ops; the attention-path error this introduces
is suppressed ~60x in the output by the residual (attention out std
~0.017 vs residual std ~1.0).

softmax: scores are built transposed (scoresT[k, q]) so probs@V needs no
transpose; the row-sum comes from a 65th all-ones column on V scaled by
exp(mask)/64, which makes reciprocal(rowsum) directly produce the x64
scaling that keeps fp8 ctx in the e4m3 normal range.
"""

import os
import sys
import contextlib
import ctypes
import types

import numpy as np
import ml_dtypes

N_CORES = 8
B, S, H = 4, 2048, 1024
NH, DH = 16, 64
R = B * S            # 8192 flattened rows
RS = R // N_CORES    # 1024 rows per core (output shard)
NCH = H // 128       # 8 contraction chunks of 128
NP = NH // 2         # 8 head pairs
NKB = S // 128       # 16 key blocks
NKP = NKB // 2       # 8 key-block pairs
NQG = 2              # query groups per core
QW = RS // NQG       # 512 queries per group
VW = DH + 1          # va width: 64 dims + rowsum ones column
LN_EPS = 1e-12

last_exec_time_ns = None

# ---------------------------------------------------------------------------
# NTFF profile hook shim (axon images without antenv.axon_hooks).
_SO_PATH = "/opt/axon/libaxon_pjrt.so"


def _install_ntff_shim():
    try:
        from antenv import axon_hooks  # noqa: F401
        return
    except ImportError:
        pass
    hook = None
    try:
        lib = ctypes.CDLL(_SO_PATH)
        if hasattr(lib, "axon_start_nrt_profile"):
            lib.axon_start_nrt_profile.argtypes = [
                ctypes.POINTER(ctypes.c_int64), ctypes.c_size_t]
            lib.axon_start_nrt_profile.restype = ctypes.c_int64
            lib.axon_stop_nrt_profile.argtypes = [ctypes.c_char_p]
            lib.axon_stop_nrt_profile.restype = ctypes.c_int64

            @contextlib.contextmanager
            def _hook(output_dir, device_ids):
                import jax
                jax.devices()
                if device_ids:
                    ids = (ctypes.c_int64 * len(device_ids))(*device_ids)
                    rc = lib.axon_start_nrt_profile(ids, len(device_ids))
                else:
                    rc = lib.axon_start_nrt_profile(None, 0)
                if rc != 0:
                    raise RuntimeError(f"axon_start_nrt_profile rc={rc}")
                try:
                    yield
                finally:
                    n = lib.axon_stop_nrt_profile(str(output_dir).encode())
                    print(f"profile: {n} ntff file(s) in {output_dir}",
                          file=sys.stderr)

            hook = _hook
    except OSError:
        pass
    mod = types.ModuleType("antenv.axon_hooks")
    mod._hook = hook
    mod.get_axon_ntff_profile_hook = lambda: mod._hook
    mod.set_axon_ntff_profile_hook = lambda h: setattr(mod, "_hook", h)
    sys.modules["antenv.axon_hooks"] = mod
    try:
        import antenv
        antenv.axon_hooks = mod
    except ImportError:
        pass


# ---------------------------------------------------------------------------

def _build(apply_gb=True):
    from concourse import bacc, tile
    import concourse.mybir as mybir

    f32 = mybir.dt.float32
    bf16 = mybir.dt.bfloat16
    fp8 = mybir.dt.float8e4
    AF = mybir.ActivationFunctionType
    ALU = mybir.AluOpType
    DR = mybir.MatmulPerfMode.DoubleRow

    nc = bacc.Bacc("TRN2", target_bir_lowering=False, debug=False,
                   num_devices=N_CORES)

    # ---- DRAM I/O (per core; b = batch, rows = this core's 1024) ----
    hT_d = nc.dram_tensor("hT", [H, S], fp8, kind="ExternalInput")
    wq_d = nc.dram_tensor("wq", [H, H], fp8, kind="ExternalInput")
    wk_d = nc.dram_tensor("wk", [H, H], fp8, kind="ExternalInput")
    wv_d = nc.dram_tensor("wv", [H, H], fp8, kind="ExternalInput")
    wo_d = nc.dram_tensor("wo", [H, H], fp8, kind="ExternalInput")
    bq_d = nc.dram_tensor("bq", [128, NP], f32, kind="ExternalInput")
    bk_d = nc.dram_tensor("bk", [128, NP], f32, kind="ExternalInput")
    bv_d = nc.dram_tensor("bv", [H], f32, kind="ExternalInput")
    if apply_gb:
        gamma_d = nc.dram_tensor("gamma", [H], f32, kind="ExternalInput")
        beta_d = nc.dram_tensor("beta", [H], f32, kind="ExternalInput")
    maskT_d = nc.dram_tensor("maskT", [128, NKB], f32, kind="ExternalInput")
    hres_d = nc.dram_tensor("hres", [RS, H], bf16, kind="ExternalInput")
    out_d = nc.dram_tensor("out", [RS, H], f32, kind="ExternalOutput")

    with tile.TileContext(nc) as tc:
        with (
            tc.tile_pool(name="const", bufs=1) as cpool,
            tc.tile_pool(name="psP", bufs=2, space="PSUM") as psP,
            tc.tile_pool(name="psS", bufs=2, space="PSUM") as psS,
            tc.tile_pool(name="psC", bufs=1, space="PSUM") as psC,
        ):
            # ================= persistent SBUF =================
            # DMAs ordered by first need: the first K/V/Q projections touch
            # only the first key/query columns, so those column slices land
            # first and compute starts ~10us in.
            wk_sb = cpool.tile([128, NCH, H], fp8, tag="wk")
            wv_sb = cpool.tile([128, NCH, H], fp8, tag="wv")
            wq_sb = cpool.tile([128, NCH, H], fp8, tag="wq")
            hT_sb = cpool.tile([128, NCH, S], fp8, tag="hT")
            bq_sb = cpool.tile([128, NP], f32, tag="bq")
            bk_sb = cpool.tile([128, NP], f32, tag="bk")
            bv_b = cpool.tile([128, H], f32, tag="bv_b")
            em_sb = cpool.tile([128, NKB], f32, tag="em")
            em64_sb = cpool.tile([128, NKB], f32, tag="em64")
            zero_h = cpool.tile([128, NH], f32, tag="zero_h")

            nc.sync.dma_start(em_sb[:, :], maskT_d[:, :])
            nc.sync.dma_start(bq_sb[:, :], bq_d[:, :])
            nc.sync.dma_start(bk_sb[:, :], bk_d[:, :])
            nc.sync.dma_start(bv_b[:, :],
                              bv_d[:].unsqueeze(0).partition_broadcast(128))
            for c in range(NCH):  # hT keys 0:512 (first K/V blocks + Q rows)
                nc.sync.dma_start(hT_sb[:, c, 0:512],
                                  hT_d[128 * c:128 * (c + 1), 0:512])
            for c in range(NCH):  # wk cols 0:512 (pairs 0-3)
                nc.sync.dma_start(wk_sb[:, c, 0:512],
                                  wk_d[128 * c:128 * (c + 1), 0:512])
            for c in range(NCH):  # wv cols 0:512 (heads 0-7)
                nc.sync.dma_start(wv_sb[:, c, 0:512],
                                  wv_d[128 * c:128 * (c + 1), 0:512])
            for c in range(NCH):  # wq cols 0:512 (pairs 0-3)
                nc.sync.dma_start(wq_sb[:, c, 0:512],
                                  wq_d[128 * c:128 * (c + 1), 0:512])
            for c in range(NCH):
                nc.sync.dma_start(hT_sb[:, c, 512:2048],
                                  hT_d[128 * c:128 * (c + 1), 512:2048])
            for c in range(NCH):
                nc.sync.dma_start(wk_sb[:, c, 512:1024],
                                  wk_d[128 * c:128 * (c + 1), 512:1024])
                nc.sync.dma_start(wq_sb[:, c, 512:1024],
                                  wq_d[128 * c:128 * (c + 1), 512:1024])
            for c in range(NCH):
                nc.sync.dma_start(wv_sb[:, c, 512:1024],
                                  wv_d[128 * c:128 * (c + 1), 512:1024])

            # exp(mask) per (key-in-block, kblock); em64 = em/64 feeds the
            # rowsum ones-column so 1/rowsum lands pre-scaled by 64.
            nc.scalar.activation(em_sb[:, :], em_sb[:, :], AF.Exp)
            nc.vector.tensor_scalar_mul(em64_sb[:, :], em_sb[:, :],
                                        float(1.0 / 64.0))
            nc.vector.memset(zero_h[:, :], 0.0)

            # big persistent activations
            kt_sb = cpool.tile([128, NP, S], bf16, tag="kt")
            qt_sb = cpool.tile([128, NP, RS], bf16, tag="qt")
            va_sb = cpool.tile([128, NKP, 2, NH, VW], fp8, tag="va")
            ctx_sb = cpool.tile([128, NCH, RS], fp8, tag="ctx")

            # out-proj phase params (DMAs overlap the early compute)
            wo_sb = cpool.tile([128, NCH, H], fp8, tag="wo")
            hres_sb = cpool.tile([128, RS // 128, H], bf16, tag="hres")
            for c in range(NCH):
                nc.sync.dma_start(wo_sb[:, c, :], wo_d[128 * c:128 * (c + 1), :])
            # hres already carries +bo (folded on host)
            for t in range(RS // 128):
                nc.sync.dma_start(hres_sb[:, t, :],
                                  hres_d[128 * t:128 * (t + 1), :])
            if apply_gb:
                gamma_b = cpool.tile([128, H], f32, tag="gamma_b")
                beta_b = cpool.tile([128, H], f32, tag="beta_b")
                nc.sync.dma_start(
                    gamma_b[:, :],
                    gamma_d[:].unsqueeze(0).partition_broadcast(128))
                nc.sync.dma_start(
                    beta_b[:, :],
                    beta_d[:].unsqueeze(0).partition_broadcast(128))

            with (
                tc.tile_pool(name="ptp", bufs=4) as ptpool,
                tc.tile_pool(name="work", bufs=2) as wpool,
            ):
                # ---------- projection emitters (JIT, woven into the
                # attention stream so ScalarE never starves) ----------
                def k_proj(p, kq):  # 512-key block kq of pair p
                    k0 = 512 * kq
                    pk = psP.tile([128, 512], f32, tag="proj")
                    for i in range(4):
                        nc.tensor.matmul(
                            pk[:, :],
                            wk_sb[:, 2 * i:2 * i + 2, 128 * p:128 * (p + 1)],
                            hT_sb[:, 2 * i:2 * i + 2, k0:k0 + 512],
                            start=(i == 0), stop=(i == 3), perf_mode=DR)
                    nc.vector.tensor_scalar(
                        kt_sb[:, p, k0:k0 + 512], pk[:, :],
                        bk_sb[:, p:p + 1], float(1.0 / 64.0),
                        ALU.add, ALU.mult)

                def q_proj(p, qg):
                    pq = psP.tile([128, 512], f32, tag="proj")
                    for i in range(4):
                        nc.tensor.matmul(
                            pq[:, :],
                            wq_sb[:, 2 * i:2 * i + 2, 128 * p:128 * (p + 1)],
                            hT_sb[:, 2 * i:2 * i + 2, QW * qg:QW * (qg + 1)],
                            start=(i == 0), stop=(i == 3), perf_mode=DR)
                    nc.vector.tensor_scalar(
                        qt_sb[:, p, QW * qg:QW * (qg + 1)], pq[:, :],
                        bq_sb[:, p:p + 1], float(1.0 / 64.0),
                        ALU.add, ALU.mult)

                def v_proj(kb, g):  # heads 8g..8g+7 of key block kb
                    kbp, sl = divmod(kb, 2)
                    emcol = em_sb[:, kb:kb + 1]
                    pv = psP.tile([128, 512], f32, tag="proj")
                    for i in range(4):
                        nc.tensor.matmul(
                            pv[:, :],
                            hT_sb[:, 2 * i:2 * i + 2,
                                  128 * kb:128 * (kb + 1)],
                            wv_sb[:, 2 * i:2 * i + 2,
                                  512 * g:512 * (g + 1)],
                            start=(i == 0), stop=(i == 3), perf_mode=DR)
                    t1 = wpool.tile([128, 512], f32, tag="t1")
                    nc.vector.scalar_tensor_tensor(
                        t1[:, :], pv[:, :], float(1.0 / 64.0),
                        bv_b[:, 512 * g:512 * (g + 1)], ALU.mult, ALU.add)
                    nc.vector.tensor_scalar_mul(
                        va_sb[:, kbp, sl, 8 * g:8 * (g + 1), 0:DH],
                        t1[:, :], emcol)
                    if g == 0:  # rowsum ones column (= exp(mask)/64)
                        nc.vector.tensor_scalar_add(
                            va_sb[:, kbp, sl, :, DH:DH + 1],
                            zero_h[:, :].unsqueeze(2),
                            em64_sb[:, kb:kb + 1])

                def normalize(qg, p, cps):
                    # ctx64[d, q] = ctx[d, q] * (64 / rowsum)
                    rs = wpool.tile([1, 1024], f32, tag="rs")
                    nc.vector.tensor_copy(rs[:, :], cps[DH:DH + 1, :])
                    rb = wpool.tile([DH, 1024], f32, tag="rb")
                    nc.gpsimd.partition_broadcast(rb[:, 0:512], rs[:, 0:512])
                    nc.gpsimd.partition_broadcast(rb[:, 512:1024],
                                                  rs[:, 512:1024])
                    nc.vector.reciprocal_approx_fast(rb[:, :], rb[:, :])
                    nc.vector.tensor_mul(
                        ctx_sb[0:DH, p, QW * qg:QW * (qg + 1)],
                        cps[0:DH, 0:512], rb[:, 0:512])
                    nc.vector.tensor_mul(
                        ctx_sb[DH:128, p, QW * qg:QW * (qg + 1)],
                        cps[0:DH, 512:1024], rb[:, 512:1024])

                inv_h = float(1.0 / H)

                def out_proj_steps(qg):
                    # out-proj + residual + LN for 4 row tiles, as a list of
                    # (kind, closure) steps spread through the next qg's
                    # stream.  PE steps are half-tile sized so the exp
                    # stream never waits long behind them; LN stats use
                    # E[x^2]-mu^2 with sums accumulated on the drain passes;
                    # the 1/sqrt(var) is batched per qg to avoid ACT
                    # exp<->sqrt table swaps mid-softmax.
                    nt = QW // 128
                    xs, sums = [], []
                    rv4 = wpool.tile([128, nt], f32, tag="rv4")
                    nm4 = wpool.tile([128, nt], f32, tag="nm4")
                    rstd4 = wpool.tile([128, nt], f32, tag="rstd4")

                    def mm_step(tt, g):
                        t = nt * qg + tt
                        if g == 0:
                            xs.append(wpool.tile([128, H], f32, tag="xln",
                                                 bufs=nt, name="x_sb"))
                            sums.append([None, None])
                        x_sb = xs[tt]
                        po = psP.tile([128, 512], f32, tag="proj")
                        for i in range(4):
                            nc.tensor.matmul(
                                po[:, :],
                                ctx_sb[:, 2 * i:2 * i + 2,
                                       128 * t:128 * (t + 1)],
                                wo_sb[:, 2 * i:2 * i + 2,
                                      512 * g:512 * (g + 1)],
                                start=(i == 0), stop=(i == 3), perf_mode=DR)
                        acc = wpool.tile([128, 1], f32, tag="acc", bufs=8,
                                         name="acc")
                        sums[tt][g] = acc
                        nc.vector.scalar_tensor_tensor(
                            x_sb[:, 512 * g:512 * (g + 1)], po[:, :],
                            float(1.0 / 4096.0),
                            hres_sb[:, t, 512 * g:512 * (g + 1)],
                            ALU.mult, ALU.add, accum_out=acc[:, :])

                    def stat_step(tt):
                        x_sb = xs[tt]
                        # negmu = -(sum0+sum1)/H
                        ssum = wpool.tile([128, 1], f32, tag="ssum")
                        nc.vector.tensor_add(ssum[:, :], sums[tt][0][:, :],
                                             sums[tt][1][:, :])
                        nc.vector.tensor_scalar_mul(nm4[:, tt:tt + 1],
                                                    ssum[:, :], -inv_h)
                        # ssq = sum(x^2); var = ssq/H - mu^2 + eps
                        xsq = wpool.tile([128, H], f32, tag="xsq")
                        ssq = wpool.tile([128, 1], f32, tag="ssq")
                        nc.vector.scalar_tensor_tensor(
                            xsq[:, :], x_sb[:, :], 1.0, x_sb[:, :],
                            ALU.mult, ALU.mult, accum_out=ssq[:, :])
                        musq = wpool.tile([128, 1], f32, tag="musq")
                        nc.vector.tensor_mul(musq[:, :], nm4[:, tt:tt + 1],
                                             nm4[:, tt:tt + 1])
                        var = wpool.tile([128, 1], f32, tag="var")
                        nc.vector.tensor_scalar(var[:, :], ssq[:, :], inv_h,
                                                LN_EPS, ALU.mult, ALU.add)
                        nc.vector.tensor_sub(var[:, :], var[:, :],
                                             musq[:, :])
                        nc.vector.reciprocal(rv4[:, tt:tt + 1], var[:, :])

                    def sqrt_step():
                        nc.scalar.activation(rstd4[:, :], rv4[:, :], AF.Sqrt)

                    def fin_step(tt):
                        t = nt * qg + tt
                        # y = x*rstd + (-mu*rstd)   [ * gamma + beta ]
                        nmr = wpool.tile([128, 1], f32, tag="nmr")
                        nc.vector.tensor_mul(nmr[:, :], nm4[:, tt:tt + 1],
                                             rstd4[:, tt:tt + 1])
                        y_sb = wpool.tile([128, H], f32, tag="xsq")
                        nc.vector.tensor_scalar(
                            y_sb[:, :], xs[tt][:, :], rstd4[:, tt:tt + 1],
                            nmr[:, :], ALU.mult, ALU.add)
                        if apply_gb:
                            nc.vector.tensor_mul(y_sb[:, :], y_sb[:, :],
                                                 gamma_b[:, :])
                            nc.vector.tensor_add(y_sb[:, :], y_sb[:, :],
                                                 beta_b[:, :])
                        nc.sync.dma_start(out_d[128 * t:128 * (t + 1), :],
                                          y_sb[:, :])

                    steps = []
                    for tt in range(nt):
                        steps.append(('pe', lambda tt=tt: mm_step(tt, 0)))
                        steps.append(('pe', lambda tt=tt: mm_step(tt, 1)))
                        steps.append(('dve', lambda tt=tt: stat_step(tt)))
                    steps.append(('dve', sqrt_step))
                    for tt in range(nt):
                        steps.append(('dve', lambda tt=tt: fin_step(tt)))
                    return steps

                # ---------- flattened attention stream ----------
                # pv runs two (qg,p,kbp) iterations behind scores/exp so
                # the exp stream never waits on the pair-boundary
                # normalize -> ctx-bank-reuse chain.
                pending_pv = []      # [(qg, p, kbp, pt), ...]
                out_steps = []       # deferred out-proj work
                cps_holder = [None]

                def flush_pv(lag):
                    while len(pending_pv) > lag:
                        fqg, fp, fkbp, fpt = pending_pv.pop(0)
                        if fkbp == 0:
                            cps_holder[0] = psC.tile([128, 1024], f32,
                                                     tag="ctxp", name="cps")
                        cps = cps_holder[0]
                        for h in range(2):
                            nc.tensor.matmul(
                                cps[0:VW, 512 * h:512 * h + 512],
                                va_sb[:, fkbp, :, 2 * fp + h, :],
                                fpt[:, :, h, :],
                                start=(fkbp == 0), stop=(fkbp == NKP - 1),
                                perf_mode=DR)
                        if fkbp == NKP - 1:
                            normalize(fqg, fp, cps)

                def pop_step(want_pe_budget):
                    # emit at most one PE step (plus any number of leading
                    # DVE-only steps, which never block the PE FIFO)
                    while out_steps:
                        kind, fn = out_steps[0]
                        if kind == 'pe' and not want_pe_budget:
                            return
                        out_steps.pop(0)
                        fn()
                        if kind == 'pe':
                            return

                for qg in range(NQG):
                    for p in range(NP):
                        if qg == 0:
                            if p > 0:
                                for kq in range(4):
                                    k_proj(p, kq)
                            q_proj(p, 0)
                        else:
                            q_proj(p, 1)
                        for kbp in range(NKP):
                            if qg == 0 and p == 0:
                                if kbp % 2 == 0:
                                    k_proj(0, kbp // 2)
                                v_proj(2 * kbp, 0)
                                v_proj(2 * kbp + 1, 0)
                            if qg == 0 and p == 4:
                                v_proj(2 * kbp, 1)
                                v_proj(2 * kbp + 1, 1)
                            pt = ptpool.tile([128, 2, 2, 512], fp8, tag="pt")
                            for sl in range(2):
                                kb = 2 * kbp + sl
                                sc = psS.tile([128, 2, 512], f32, tag="sc")
                                nc.tensor.matmul(
                                    sc[:, 0, :],
                                    kt_sb[0:DH, p, 128 * kb:128 * (kb + 1)],
                                    qt_sb[0:DH, p, QW * qg:QW * (qg + 1)],
                                    start=True, stop=True)
                                nc.tensor.matmul(
                                    sc[:, 1, :],
                                    kt_sb[DH:128, p, 128 * kb:128 * (kb + 1)],
                                    qt_sb[DH:128, p, QW * qg:QW * (qg + 1)],
                                    start=True, stop=True)
                                nc.scalar.activation(
                                    pt[:, sl, :, :], sc[:, :, :],
                                    AF.Exp, scale=0.125)
                            flush_pv(lag=2)
                            pending_pv.append((qg, p, kbp, pt))
                            pop_step(kbp % 2 == 1
                                     and not (qg == 0 and p in (0, 4)))
                    # defer this qg's out-proj into the next qg's stream;
                    # drain the pv pipeline first so every ctx column this
                    # qg produced has its normalize emitted before any
                    # out-proj step can read it
                    flush_pv(lag=0)
                    out_steps += out_proj_steps(qg)
                while out_steps:
                    out_steps.pop(0)[1]()

    nc.compile()
    return nc


_NC_CACHE = {}


def _get_nc(apply_gb):
    if apply_gb not in _NC_CACHE:
        _NC_CACHE[apply_gb] = _build(apply_gb)
    return _NC_CACHE[apply_gb]


def _make_in_maps(hidden_states, attention_mask, Wq, bq, Wk, bk, Wv, bv, Wo,
                  bo, ln_gamma, ln_beta, apply_gb):
    f8 = ml_dtypes.float8_e4m3fn
    hid2 = np.asarray(hidden_states, np.float32).reshape(R, H)
    wq8 = np.ascontiguousarray(np.asarray(Wq, np.float32).T * 64.0).astype(f8)
    wk8 = np.ascontiguousarray(np.asarray(Wk, np.float32).T * 64.0).astype(f8)
    wv8 = np.ascontiguousarray(np.asarray(Wv, np.float32).T * 64.0).astype(f8)
    wo8 = np.ascontiguousarray(np.asarray(Wo, np.float32).T * 64.0).astype(f8)
    bq2 = np.ascontiguousarray(
        (np.asarray(bq, np.float32) * 64.0).reshape(NP, 128).T)
    bk2 = np.ascontiguousarray(
        (np.asarray(bk, np.float32) * 64.0).reshape(NP, 128).T)
    bv32 = np.asarray(bv, np.float32)
    bo32 = np.asarray(bo, np.float32)
    gamma32 = np.asarray(ln_gamma, np.float32)
    beta32 = np.asarray(ln_beta, np.float32)
    mask2 = np.asarray(attention_mask, np.float32).reshape(B, S)

    in_maps = []
    for c in range(N_CORES):
        b, half = divmod(c, 2)
        hb = hid2[S * b:S * (b + 1), :]          # this batch's rows [2048, H]
        # roll so this core's own 1024 rows sit in columns [0, 1024) of hT
        hb_roll = np.roll(hb, -RS * half, axis=0)
        hT8 = np.ascontiguousarray(hb_roll.T).astype(f8)
        # mask key order must match the rolled key order
        m_roll = np.roll(mask2[b], -RS * half)
        maskT = np.ascontiguousarray(m_roll.reshape(NKB, 128).T)
        m = {
            "hT": hT8,
            "wq": wq8, "wk": wk8, "wv": wv8, "wo": wo8,
            "bq": bq2, "bk": bk2, "bv": bv32,
            "maskT": maskT,
            "hres": np.ascontiguousarray(
                hid2[RS * c:RS * (c + 1), :] + bo32[None, :]
            ).astype(ml_dtypes.bfloat16),
        }
        if apply_gb:
            m["gamma"] = gamma32
            m["beta"] = beta32
        in_maps.append(m)
    return in_maps


def kernel(hidden_states, attention_mask, Wq, bq, Wk, bk, Wv, bv, Wo, bo,
           ln_gamma, ln_beta):
    global last_exec_time_ns
    from concourse.bass_utils import run_bass_kernel_spmd

    _install_ntff_shim()
    apply_gb = not (np.all(np.asarray(ln_gamma) == 1.0)
                    and np.all(np.asarray(ln_beta) == 0.0))
    in_maps = _make_in_maps(hidden_states, attention_mask, Wq, bq, Wk, bk,
                            Wv, bv, Wo, bo, ln_gamma, ln_beta, apply_gb)
    nc = _get_nc(apply_gb)
    trace = os.environ.get("BASS_KERNEL_TRACE", "0") == "1"
    res = run_bass_kernel_spmd(nc, in_maps, core_ids=list(range(N_CORES)),
                               trace=trace)
    last_exec_time_ns = res.exec_time_ns
    if trace and res.exec_time_ns is not None:
        print(f"HW exec time: {res.exec_time_ns} ns")

    out = np.concatenate([res.results[c]["out"] for c in range(N_CORES)],
                         axis=0)
    return out.reshape(B, S, H).astype(np.float32)


# revision 32
# speedup vs baseline: 1.5667x; 1.0094x over previous
"""BertAttention (QKV proj + MHA + out-proj + residual + LayerNorm) on 8
Trainium2 NeuronCores -- fully local, zero-collective version.

Sharding: each core owns a 1024-row shard of the flattened (B*S, H)
output: core c -> batch b=c//2, seq half c%2.  The core computes K/V
projections for its WHOLE batch (2048 keys, all 16 heads; K/V proj is
duplicated across the 2 cores of a batch -- cheaper than the AllToAll it
replaces), Q projection for its own 1024 rows, attention for all 16
heads over its rows, then output projection + residual + LayerNorm with
a fully local contraction.  No cross-device traffic at all.

Precision: fp8(e4m3) + DoubleRow (2 key-blocks / 2 contraction chunks
per matmul) for the K/V/Q/out projections and the probs@V contraction;
bf16 for the score matmuls (contraction DH=64, two heads row-tiled into
the 128-row PE array concurrently); fp32 softmax statistics, residual
accumulate in fp32 from a bf16 residual, LayerNorm fp32.  fp8 operands
with small magnitudes are pre-scaled by 64 on the host (weights) and
rescaled in the PSUM-drain ops; the attention-path error this introduces
is suppressed ~60x in the output by the residual (attention out std
~0.017 vs residual std ~1.0).

softmax: scores are built transposed (scoresT[k, q]) so probs@V needs no
transpose; the row-sum comes from a 65th all-ones column on V scaled by
exp(mask)/64, which makes reciprocal(rowsum) directly produce the x64
scaling that keeps fp8 ctx in the e4m3 normal range.
"""

import os
import sys
import contextlib
import ctypes
import types

import numpy as np
import ml_dtypes

N_CORES = 8
B, S, H = 4, 2048, 1024
NH, DH = 16, 64
R = B * S            # 8192 flattened rows
RS = R // N_CORES    # 1024 rows per core (output shard)
NCH = H // 128       # 8 contraction chunks of 128
NP = NH // 2         # 8 head pairs
NKB = S // 128       # 16 key blocks
NKP = NKB // 2       # 8 key-block pairs
NQG = 2              # query groups per core
QW = RS // NQG       # 512 queries per group
VW = DH + 1          # va width: 64 dims + rowsum ones column
LN_EPS = 1e-12

last_exec_time_ns = None

# ---------------------------------------------------------------------------
# NTFF profile hook shim (axon images without antenv.axon_hooks).
_SO_PATH = "/opt/axon/libaxon_pjrt.so"


def _install_ntff_shim():
    try:
        from antenv import axon_hooks  # noqa: F401
        return
    except ImportError:
        pass
    hook = None
    try:
        lib = ctypes.CDLL(_SO_PATH)
        if hasattr(lib, "axon_start_nrt_profile"):
            lib.axon_start_nrt_profile.argtypes = [
                ctypes.POINTER(ctypes.c_int64), ctypes.c_size_t]
            lib.axon_start_nrt_profile.restype = ctypes.c_int64
            lib.axon_stop_nrt_profile.argtypes = [ctypes.c_char_p]
            lib.axon_stop_nrt_profile.restype = ctypes.c_int64

            @contextlib.contextmanager
            def _hook(output_dir, device_ids):
                import jax
                jax.devices()
                if device_ids:
                    ids = (ctypes.c_int64 * len(device_ids))(*device_ids)
                    rc = lib.axon_start_nrt_profile(ids, len(device_ids))
                else:
                    rc = lib.axon_start_nrt_profile(None, 0)
                if rc != 0:
                    raise RuntimeError(f"axon_start_nrt_profile rc={rc}")
                try:
                    yield
                finally:
                    n = lib.axon_stop_nrt_profile(str(output_dir).encode())
                    print(f"profile: {n} ntff file(s) in {output_dir}",
                          file=sys.stderr)

            hook = _hook
    except OSError:
        pass
    mod = types.ModuleType("antenv.axon_hooks")
    mod._hook = hook
    mod.get_axon_ntff_profile_hook = lambda: mod._hook
    mod.set_axon_ntff_profile_hook = lambda h: setattr(mod, "_hook", h)
    sys.modules["antenv.axon_hooks"] = mod
    try:
        import antenv
        antenv.axon_hooks = mod
    except ImportError:
        pass


# ---------------------------------------------------------------------------

def _build(apply_gb=True):
    from concourse import bacc, tile
    import concourse.mybir as mybir

    f32 = mybir.dt.float32
    bf16 = mybir.dt.bfloat16
    fp8 = mybir.dt.float8e4
    AF = mybir.ActivationFunctionType
    ALU = mybir.AluOpType
    DR = mybir.MatmulPerfMode.DoubleRow

    nc = bacc.Bacc("TRN2", target_bir_lowering=False, debug=False,
                   num_devices=N_CORES)

    # ---- DRAM I/O (per core; b = batch, rows = this core's 1024) ----
    hT_d = nc.dram_tensor("hT", [H, S], fp8, kind="ExternalInput")
    wq_d = nc.dram_tensor("wq", [H, H], fp8, kind="ExternalInput")
    wk_d = nc.dram_tensor("wk", [H, H], fp8, kind="ExternalInput")
    wv_d = nc.dram_tensor("wv", [H, H], fp8, kind="ExternalInput")
    wo_d = nc.dram_tensor("wo", [H, H], fp8, kind="ExternalInput")
    bq_d = nc.dram_tensor("bq", [128, NP], f32, kind="ExternalInput")
    bk_d = nc.dram_tensor("bk", [128, NP], f32, kind="ExternalInput")
    bv_d = nc.dram_tensor("bv", [H], f32, kind="ExternalInput")
    if apply_gb:
        gamma_d = nc.dram_tensor("gamma", [H], f32, kind="ExternalInput")
        beta_d = nc.dram_tensor("beta", [H], f32, kind="ExternalInput")
    maskT_d = nc.dram_tensor("maskT", [128, NKB], f32, kind="ExternalInput")
    hres_d = nc.dram_tensor("hres", [RS, H], bf16, kind="ExternalInput")
    out_d = nc.dram_tensor("out", [RS, H], f32, kind="ExternalOutput")

    with tile.TileContext(nc) as tc:
        with (
            tc.tile_pool(name="const", bufs=1) as cpool,
            tc.tile_pool(name="psP", bufs=2, space="PSUM") as psP,
            tc.tile_pool(name="psS", bufs=2, space="PSUM") as psS,
            tc.tile_pool(name="psC", bufs=1, space="PSUM") as psC,
        ):
            # ================= persistent SBUF =================
            # DMAs ordered by first need: the first K/V/Q projections touch
            # only the first key/query columns, so those column slices land
            # first and compute starts ~10us in.
            wk_sb = cpool.tile([128, NCH, H], fp8, tag="wk")
            wv_sb = cpool.tile([128, NCH, H], fp8, tag="wv")
            wq_sb = cpool.tile([128, NCH, H], fp8, tag="wq")
            hT_sb = cpool.tile([128, NCH, S], fp8, tag="hT")
            bq_sb = cpool.tile([128, NP], f32, tag="bq")
            bk_sb = cpool.tile([128, NP], f32, tag="bk")
            bv_b = cpool.tile([128, H], f32, tag="bv_b")
            em_sb = cpool.tile([128, NKB], f32, tag="em")
            em64_sb = cpool.tile([128, NKB], f32, tag="em64")
            zero_h = cpool.tile([128, NH], f32, tag="zero_h")

            # DMA issue costs ~650ns of queue time apiece, so the loads are
            # spread across all five engine queues and ordered by first
            # need (first K/V/Q projections touch only the first columns).
            nc.scalar.dma_start(em_sb[:, :], maskT_d[:, :])
            nc.sync.dma_start(bq_sb[:, :], bq_d[:, :])
            nc.sync.dma_start(bk_sb[:, :], bk_d[:, :])
            nc.sync.dma_start(bv_b[:, :],
                              bv_d[:].unsqueeze(0).partition_broadcast(128))
            for c in range(NCH):  # hT keys 0:512 (first K/V blocks + Q rows)
                nc.scalar.dma_start(hT_sb[:, c, 0:512],
                                    hT_d[128 * c:128 * (c + 1), 0:512])
            for c in range(NCH):  # wk cols 0:512 (pairs 0-3)
                nc.sync.dma_start(wk_sb[:, c, 0:512],
                                  wk_d[128 * c:128 * (c + 1), 0:512])
            for c in range(NCH):  # wv cols 0:512 (heads 0-7)
                nc.scalar.dma_start(wv_sb[:, c, 0:512],
                                    wv_d[128 * c:128 * (c + 1), 0:512])
            # exp(mask) per (key-in-block, kblock); em64 = em/64 feeds the
            # rowsum ones-column so 1/rowsum lands pre-scaled by 64.
            nc.scalar.activation(em_sb[:, :], em_sb[:, :], AF.Exp)
            nc.vector.tensor_scalar_mul(em64_sb[:, :], em_sb[:, :],
                                        float(1.0 / 64.0))
            nc.vector.memset(zero_h[:, :], 0.0)

            for c in range(NCH):  # wq cols 0:512 (pairs 0-3)
                nc.sync.dma_start(wq_sb[:, c, 0:512],
                                  wq_d[128 * c:128 * (c + 1), 0:512])
            for c in range(NCH):  # hT keys 512:1024 (K blocks 2-3 of p0)
                nc.gpsimd.dma_start(hT_sb[:, c, 512:1024],
                                    hT_d[128 * c:128 * (c + 1), 512:1024])
            for c in range(NCH):
                nc.gpsimd.dma_start(hT_sb[:, c, 1024:1536],
                                    hT_d[128 * c:128 * (c + 1), 1024:1536])
            for c in range(NCH):
                nc.gpsimd.dma_start(hT_sb[:, c, 1536:2048],
                                    hT_d[128 * c:128 * (c + 1), 1536:2048])
            for c in range(NCH):
                nc.sync.dma_start(wq_sb[:, c, 512:1024],
                                  wq_d[128 * c:128 * (c + 1), 512:1024])
            for c in range(NCH):
                nc.sync.dma_start(wk_sb[:, c, 512:1024],
                                  wk_d[128 * c:128 * (c + 1), 512:1024])
            for c in range(NCH):
                nc.scalar.dma_start(wv_sb[:, c, 512:1024],
                                    wv_d[128 * c:128 * (c + 1), 512:1024])

            # big persistent activations
            kt_sb = cpool.tile([128, NP, S], bf16, tag="kt")
            qt_sb = cpool.tile([128, NP, RS], bf16, tag="qt")
            va_sb = cpool.tile([128, NKP, 2, NH, VW], fp8, tag="va")
            ctx_sb = cpool.tile([128, NCH, RS], fp8, tag="ctx")

            # out-proj phase params (DMAs overlap the early compute)
            wo_sb = cpool.tile([128, NCH, H], fp8, tag="wo")
            hres_sb = cpool.tile([128, RS // 128, H], bf16, tag="hres")
            for c in range(NCH):
                nc.gpsimd.dma_start(wo_sb[:, c, :],
                                    wo_d[128 * c:128 * (c + 1), :])
            # hres already carries +bo (folded on host)
            for t in range(RS // 128):
                nc.gpsimd.dma_start(hres_sb[:, t, :],
                                    hres_d[128 * t:128 * (t + 1), :])
            if apply_gb:
                gamma_b = cpool.tile([128, H], f32, tag="gamma_b")
                beta_b = cpool.tile([128, H], f32, tag="beta_b")
                nc.gpsimd.dma_start(
                    gamma_b[:, :],
                    gamma_d[:].unsqueeze(0).partition_broadcast(128))
                nc.gpsimd.dma_start(
                    beta_b[:, :],
                    beta_d[:].unsqueeze(0).partition_broadcast(128))

            with (
                tc.tile_pool(name="ptp", bufs=5) as ptpool,
                tc.tile_pool(name="work", bufs=2) as wpool,
            ):
                # ---------- projection emitters (JIT, woven into the
                # attention stream so ScalarE never starves) ----------
                def k_proj(p, kq):  # 512-key block kq of pair p
                    k0 = 512 * kq
                    pk = psP.tile([128, 512], f32, tag="proj")
                    for i in range(4):
                        nc.tensor.matmul(
                            pk[:, :],
                            wk_sb[:, 2 * i:2 * i + 2, 128 * p:128 * (p + 1)],
                            hT_sb[:, 2 * i:2 * i + 2, k0:k0 + 512],
                            start=(i == 0), stop=(i == 3), perf_mode=DR)
                    nc.vector.tensor_scalar(
                        kt_sb[:, p, k0:k0 + 512], pk[:, :],
                        bk_sb[:, p:p + 1], float(1.0 / 64.0),
                        ALU.add, ALU.mult)

                def q_proj(p, qg):
                    pq = psP.tile([128, 512], f32, tag="proj")
                    for i in range(4):
                        nc.tensor.matmul(
                            pq[:, :],
                            wq_sb[:, 2 * i:2 * i + 2, 128 * p:128 * (p + 1)],
                            hT_sb[:, 2 * i:2 * i + 2, QW * qg:QW * (qg + 1)],
                            start=(i == 0), stop=(i == 3), perf_mode=DR)
                    nc.vector.tensor_scalar(
                        qt_sb[:, p, QW * qg:QW * (qg + 1)], pq[:, :],
                        bq_sb[:, p:p + 1], float(1.0 / 64.0),
                        ALU.add, ALU.mult)

                def v_proj(kb, g):  # heads 8g..8g+7 of key block kb
                    kbp, sl = divmod(kb, 2)
                    emcol = em_sb[:, kb:kb + 1]
                    pv = psP.tile([128, 512], f32, tag="proj")
                    for i in range(4):
                        nc.tensor.matmul(
                            pv[:, :],
                            hT_sb[:, 2 * i:2 * i + 2,
                                  128 * kb:128 * (kb + 1)],
                            wv_sb[:, 2 * i:2 * i + 2,
                                  512 * g:512 * (g + 1)],
                            start=(i == 0), stop=(i == 3), perf_mode=DR)
                    t1 = wpool.tile([128, 512], f32, tag="t1")
                    nc.vector.scalar_tensor_tensor(
                        t1[:, :], pv[:, :], float(1.0 / 64.0),
                        bv_b[:, 512 * g:512 * (g + 1)], ALU.mult, ALU.add)
                    nc.vector.tensor_scalar_mul(
                        va_sb[:, kbp, sl, 8 * g:8 * (g + 1), 0:DH],
                        t1[:, :], emcol)
                    if g == 0:  # rowsum ones column (= exp(mask)/64)
                        nc.vector.tensor_scalar_add(
                            va_sb[:, kbp, sl, :, DH:DH + 1],
                            zero_h[:, :].unsqueeze(2),
                            em64_sb[:, kb:kb + 1])

                def normalize(qg, p, cps):
                    # ctx64[d, q] = ctx[d, q] * (64 / rowsum)
                    rs = wpool.tile([1, 1024], f32, tag="rs")
                    nc.vector.tensor_copy(rs[:, :], cps[DH:DH + 1, :])
                    rb = wpool.tile([DH, 1024], f32, tag="rb")
                    nc.gpsimd.partition_broadcast(rb[:, :], rs[:, :])
                    nc.vector.reciprocal_approx_fast(rb[:, :], rb[:, :])
                    nc.vector.tensor_mul(
                        ctx_sb[0:DH, p, QW * qg:QW * (qg + 1)],
                        cps[0:DH, 0:512], rb[:, 0:512])
                    nc.vector.tensor_mul(
                        ctx_sb[DH:128, p, QW * qg:QW * (qg + 1)],
                        cps[0:DH, 512:1024], rb[:, 512:1024])

                inv_h = float(1.0 / H)

                def out_proj_steps(qg):
                    # out-proj + residual + LN for 4 row tiles, as a list of
                    # (kind, closure) steps spread through the next qg's
                    # stream.  PE steps are half-tile sized so the exp
                    # stream never waits long behind them; LN stats use
                    # E[x^2]-mu^2 with sums accumulated on the drain passes;
                    # the 1/sqrt(var) is batched per qg to avoid ACT
                    # exp<->sqrt table swaps mid-softmax.
                    nt = QW // 128
                    xs, sums = [], []
                    rv4 = wpool.tile([128, nt], f32, tag="rv4")
                    nm4 = wpool.tile([128, nt], f32, tag="nm4")
                    rstd4 = wpool.tile([128, nt], f32, tag="rstd4")

                    def mm_step(tt, g):
                        t = nt * qg + tt
                        if g == 0:
                            xs.append(wpool.tile([128, H], f32, tag="xln",
                                                 bufs=nt, name="x_sb"))
                            sums.append([None, None])
                        x_sb = xs[tt]
                        po = psP.tile([128, 512], f32, tag="proj")
                        for i in range(4):
                            nc.tensor.matmul(
                                po[:, :],
                                ctx_sb[:, 2 * i:2 * i + 2,
                                       128 * t:128 * (t + 1)],
                                wo_sb[:, 2 * i:2 * i + 2,
                                      512 * g:512 * (g + 1)],
                                start=(i == 0), stop=(i == 3), perf_mode=DR)
                        acc = wpool.tile([128, 1], f32, tag="acc", bufs=8,
                                         name="acc")
                        sums[tt][g] = acc
                        nc.vector.scalar_tensor_tensor(
                            x_sb[:, 512 * g:512 * (g + 1)], po[:, :],
                            float(1.0 / 4096.0),
                            hres_sb[:, t, 512 * g:512 * (g + 1)],
                            ALU.mult, ALU.add, accum_out=acc[:, :])

                    def stat_step(tt):
                        x_sb = xs[tt]
                        # negmu = -(sum0+sum1)/H
                        ssum = wpool.tile([128, 1], f32, tag="ssum")
                        nc.vector.tensor_add(ssum[:, :], sums[tt][0][:, :],
                                             sums[tt][1][:, :])
                        nc.vector.tensor_scalar_mul(nm4[:, tt:tt + 1],
                                                    ssum[:, :], -inv_h)
                        # ssq = sum(x^2); var = ssq/H - mu^2 + eps
                        xsq = wpool.tile([128, H], f32, tag="xsq")
                        ssq = wpool.tile([128, 1], f32, tag="ssq")
                        nc.vector.scalar_tensor_tensor(
                            xsq[:, :], x_sb[:, :], 1.0, x_sb[:, :],
                            ALU.mult, ALU.mult, accum_out=ssq[:, :])
                        musq = wpool.tile([128, 1], f32, tag="musq")
                        nc.vector.tensor_mul(musq[:, :], nm4[:, tt:tt + 1],
                                             nm4[:, tt:tt + 1])
                        var = wpool.tile([128, 1], f32, tag="var")
                        nc.vector.tensor_scalar(var[:, :], ssq[:, :], inv_h,
                                                LN_EPS, ALU.mult, ALU.add)
                        nc.vector.tensor_sub(var[:, :], var[:, :],
                                             musq[:, :])
                        nc.vector.reciprocal(rv4[:, tt:tt + 1], var[:, :])

                    def sqrt_step():
                        nc.scalar.activation(rstd4[:, :], rv4[:, :], AF.Sqrt)

                    def fin_step(tt):
                        t = nt * qg + tt
                        # y = x*rstd + (-mu*rstd)   [ * gamma + beta ]
                        nmr = wpool.tile([128, 1], f32, tag="nmr")
                        nc.vector.tensor_mul(nmr[:, :], nm4[:, tt:tt + 1],
                                             rstd4[:, tt:tt + 1])
                        y_sb = wpool.tile([128, H], f32, tag="xsq")
                        nc.vector.tensor_scalar(
                            y_sb[:, :], xs[tt][:, :], rstd4[:, tt:tt + 1],
                            nmr[:, :], ALU.mult, ALU.add)
                        if apply_gb:
                            nc.vector.tensor_mul(y_sb[:, :], y_sb[:, :],
                                                 gamma_b[:, :])
                            nc.vector.tensor_add(y_sb[:, :], y_sb[:, :],
                                                 beta_b[:, :])
                        nc.sync.dma_start(out_d[128 * t:128 * (t + 1), :],
                                          y_sb[:, :])

                    steps = []
                    for tt in range(nt):
                        steps.append(('pe', lambda tt=tt: mm_step(tt, 0)))
                        steps.append(('pe', lambda tt=tt: mm_step(tt, 1)))
                        steps.append(('dve', lambda tt=tt: stat_step(tt)))
                    steps.append(('dve', sqrt_step))
                    for tt in range(nt):
                        steps.append(('dve', lambda tt=tt: fin_step(tt)))
                    return steps

                # ---------- flattened attention stream ----------
                # pv runs two (qg,p,kbp) iterations behind scores/exp so
                # the exp stream never waits on the pair-boundary
                # normalize -> ctx-bank-reuse chain.
                pending_pv = []      # [(qg, p, kbp, pt), ...]
                out_steps = []       # deferred out-proj work
                cps_holder = [None]

                def flush_pv(lag):
                    while len(pending_pv) > lag:
                        fqg, fp, fkbp, fpt = pending_pv.pop(0)
                        if fkbp == 0:
                            cps_holder[0] = psC.tile([128, 1024], f32,
                                                     tag="ctxp", name="cps")
                        cps = cps_holder[0]
                        for h in range(2):
                            nc.tensor.matmul(
                                cps[0:VW, 512 * h:512 * h + 512],
                                va_sb[:, fkbp, :, 2 * fp + h, :],
                                fpt[:, :, h, :],
                                start=(fkbp == 0), stop=(fkbp == NKP - 1),
                                perf_mode=DR)
                        if fkbp == NKP - 1:
                            normalize(fqg, fp, cps)

                def pop_step(want_pe_budget):
                    # emit at most one PE step (plus any number of leading
                    # DVE-only steps, which never block the PE FIFO)
                    while out_steps:
                        kind, fn = out_steps[0]
                        if kind == 'pe' and not want_pe_budget:
                            return
                        out_steps.pop(0)
                        fn()
                        if kind == 'pe':
                            return

                it = 0
                for qg in range(NQG):
                    for p in range(NP):
                        if qg == 0:
                            if p > 0:
                                for kq in range(4):
                                    k_proj(p, kq)
                            q_proj(p, 0)
                        else:
                            q_proj(p, 1)
                        for kbp in range(NKP):
                            if qg == 0 and p == 0:
                                if kbp % 2 == 0:
                                    k_proj(0, kbp // 2)
                                v_proj(2 * kbp, 0)
                                v_proj(2 * kbp + 1, 0)
                            if qg == 0 and p == 4:
                                v_proj(2 * kbp, 1)
                                v_proj(2 * kbp + 1, 1)
                            pt = ptpool.tile([128, 2, 2, 512], fp8, tag="pt")
                            for sl in range(2):
                                kb = 2 * kbp + sl
                                sc = psS.tile([128, 2, 512], f32, tag="sc")
                                nc.tensor.matmul(
                                    sc[:, 0, :],
                                    kt_sb[0:DH, p, 128 * kb:128 * (kb + 1)],
                                    qt_sb[0:DH, p, QW * qg:QW * (qg + 1)],
                                    start=True, stop=True)
                                nc.tensor.matmul(
                                    sc[:, 1, :],
                                    kt_sb[DH:128, p, 128 * kb:128 * (kb + 1)],
                                    qt_sb[DH:128, p, QW * qg:QW * (qg + 1)],
                                    start=True, stop=True)
                                nc.scalar.activation(
                                    pt[:, sl, :, :], sc[:, :, :],
                                    AF.Exp, scale=0.125)
                            flush_pv(lag=3)
                            pending_pv.append((qg, p, kbp, pt))
                            it += 1
                            pop_step(it % 3 == 2
                                     and not (qg == 0 and p in (0, 4)))
                    # defer this qg's out-proj into the next qg's stream;
                    # drain the pv pipeline first so every ctx column this
                    # qg produced has its normalize emitted before any
                    # out-proj step can read it
                    flush_pv(lag=0)
                    out_steps += out_proj_steps(qg)
                while out_steps:
                    out_steps.pop(0)[1]()

    nc.compile()
    return nc


_NC_CACHE = {}


def _get_nc(apply_gb):
    if apply_gb not in _NC_CACHE:
        _NC_CACHE[apply_gb] = _build(apply_gb)
    return _NC_CACHE[apply_gb]


def _make_in_maps(hidden_states, attention_mask, Wq, bq, Wk, bk, Wv, bv, Wo,
                  bo, ln_gamma, ln_beta, apply_gb):
    f8 = ml_dtypes.float8_e4m3fn
    hid2 = np.asarray(hidden_states, np.float32).reshape(R, H)
    wq8 = np.ascontiguousarray(np.asarray(Wq, np.float32).T * 64.0).astype(f8)
    wk8 = np.ascontiguousarray(np.asarray(Wk, np.float32).T * 64.0).astype(f8)
    wv8 = np.ascontiguousarray(np.asarray(Wv, np.float32).T * 64.0).astype(f8)
    wo8 = np.ascontiguousarray(np.asarray(Wo, np.float32).T * 64.0).astype(f8)
    bq2 = np.ascontiguousarray(
        (np.asarray(bq, np.float32) * 64.0).reshape(NP, 128).T)
    bk2 = np.ascontiguousarray(
        (np.asarray(bk, np.float32) * 64.0).reshape(NP, 128).T)
    bv32 = np.asarray(bv, np.float32)
    bo32 = np.asarray(bo, np.float32)
    gamma32 = np.asarray(ln_gamma, np.float32)
    beta32 = np.asarray(ln_beta, np.float32)
    mask2 = np.asarray(attention_mask, np.float32).reshape(B, S)

    in_maps = []
    for c in range(N_CORES):
        b, half = divmod(c, 2)
        hb = hid2[S * b:S * (b + 1), :]          # this batch's rows [2048, H]
        # roll so this core's own 1024 rows sit in columns [0, 1024) of hT
        hb_roll = np.roll(hb, -RS * half, axis=0)
        hT8 = np.ascontiguousarray(hb_roll.T).astype(f8)
        # mask key order must match the rolled key order
        m_roll = np.roll(mask2[b], -RS * half)
        maskT = np.ascontiguousarray(m_roll.reshape(NKB, 128).T)
        m = {
            "hT": hT8,
            "wq": wq8, "wk": wk8, "wv": wv8, "wo": wo8,
            "bq": bq2, "bk": bk2, "bv": bv32,
            "maskT": maskT,
            "hres": np.ascontiguousarray(
                hid2[RS * c:RS * (c + 1), :] + bo32[None, :]
            ).astype(ml_dtypes.bfloat16),
        }
        if apply_gb:
            m["gamma"] = gamma32
            m["beta"] = beta32
        in_maps.append(m)
    return in_maps


def kernel(hidden_states, attention_mask, Wq, bq, Wk, bk, Wv, bv, Wo, bo,
           ln_gamma, ln_beta):
    global last_exec_time_ns
    from concourse.bass_utils import run_bass_kernel_spmd

    _install_ntff_shim()
    apply_gb = not (np.all(np.asarray(ln_gamma) == 1.0)
                    and np.all(np.asarray(ln_beta) == 0.0))
    in_maps = _make_in_maps(hidden_states, attention_mask, Wq, bq, Wk, bk,
                            Wv, bv, Wo, bo, ln_gamma, ln_beta, apply_gb)
    nc = _get_nc(apply_gb)
    trace = os.environ.get("BASS_KERNEL_TRACE", "0") == "1"
    res = run_bass_kernel_spmd(nc, in_maps, core_ids=list(range(N_CORES)),
                               trace=trace)
    last_exec_time_ns = res.exec_time_ns
    if trace and res.exec_time_ns is not None:
        print(f"HW exec time: {res.exec_time_ns} ns")

    out = np.concatenate([res.results[c]["out"] for c in range(N_CORES)],
                         axis=0)
    return out.reshape(B, S, H).astype(np.float32)
